# revision 25
# baseline (speedup 1.0000x reference)
"""Seq2seq RNN with attention on 8 TRN2 NeuronCores.

Strategy: data-parallel over batch. B=32 -> 4 batch elements per core.

Key ideas vs a naive per-step implementation:
- The decoder's h-recurrence does not depend on attention, so the decoder
  RNN runs as a bare tanh chain and attention/context/out-projection are
  computed afterwards in large batched matmuls (per 32-step block).
- RNN steps use a slot layout in SBUF: the single fused Tanh of step s
  writes [h2_{s-2} | h1_s] directly into slot s+2; the next step's
  matmuls read the slot directly (no copies on the serial chain). Layer-2
  runs 2 steps behind layer-1 so its h1-carry matmul is off the critical
  path. Per-step critical path = 4 U-matmuls + 1 activation.
- Biases b_enc1/b_dec are folded into the embedding tables host-side;
  b_enc2 is added with a prefetched rank-1 matmul (skipped when zero).
- The full W_out.T (16MB bf16) is DMA'd into SBUF during the encoder.
  Final-projection groups (2 matmuls of [128x500] + a copy) are
  interleaved one-per-decoder-step so they execute inside the tanh-wait
  windows of the serial chain; logits are staged in 16KB/partition strips
  and stored with few large DMAs.
"""

import numpy as np

import concourse.bass as bass
import concourse.bacc as bacc
import concourse.tile as tile
from concourse import mybir
from concourse.bass_utils import run_bass_kernel_spmd
from concourse.masks import make_identity

D = 256
V = 32000
T = 128  # T_SRC == T_TGT == 128
B = 32
NCORES = 8
BL = B // NCORES  # 4 batch elements per core
KC = D // 128  # 2 d-chunks of 128
DT = mybir.dt.float32
BF = mybir.dt.bfloat16
NPBF = mybir.dt.np(BF)
AF = mybir.ActivationFunctionType
ALU = mybir.AluOpType
AX = mybir.AxisListType

TBLK = 32            # decoder timesteps per attention block
NBLK = T // TBLK     # 4 blocks
PBLK = 32            # decoder timesteps per projection token tile (128 tokens)
VC = 500             # vocab chunk per projection matmul (psum <= 2KB/part)
NVC = V // VC        # 64 chunks per token tile
STRIP = 8000         # vocab columns per logits store strip (16KB/part bf16)
NSTRIP = V // STRIP  # 4 strips per token tile

_CACHE = {}


def _build(with_b2, NLO=1):
    nc = bacc.Bacc(None)

    u_d = nc.declare_dram_parameter("u", [D, D], BF, isOutput=False)
    cwt_d = nc.declare_dram_parameter("ctx_wt", [D, D], BF, isOutput=False)
    wot_d = nc.declare_dram_parameter("w_out_t", [D, V], BF, isOutput=False)
    een_d = nc.declare_dram_parameter("e_en", [V, D], BF, isOutput=False)
    ede_d = nc.declare_dram_parameter("e_de", [V, D], BF, isOutput=False)
    b2r_d = nc.declare_dram_parameter("b2row", [1, D], BF, isOutput=False)
    si_d = nc.declare_dram_parameter("src_idx", [T, BL], mybir.dt.int32, isOutput=False)
    ti_d = nc.declare_dram_parameter("tgt_idx", [T, BL], mybir.dt.int32, isOutput=False)
    out_d = nc.declare_dram_parameter("out", [T * BL, V], BF, isOutput=True)

    with tile.TileContext(nc) as tc:
        with (
            tc.tile_pool(name="persist", bufs=1) as pp,
            tc.tile_pool(name="work", bufs=6) as wp,
            tc.tile_pool(name="strips", bufs=3) as lp,
        ):
            # ---- persistent SBUF tiles ----
            u_sb = pp.tile([128, KC, D], BF, tag="u")
            cwt_sb = pp.tile([128, KC, D], BF, tag="cwt")
            w_sb = pp.tile([128, KC, V], BF, tag="w")  # 128KB/partition
            ident = pp.tile([128, 128], DT, tag="ident")
            identb = pp.tile([128, 128], BF, tag="identb")
            ones1 = pp.tile([1, 128], BF, tag="ones1")
            b2row = pp.tile([1, D], BF, tag="b2row")
            si_sb = pp.tile([T, BL], mybir.dt.int32, tag="si")
            ti_sb = pp.tile([T, BL], mybir.dt.int32, tag="ti")
            maddb = pp.tile([1, BL, T], BF, tag="maddb")
            # encoder slots: he[:, k, s, 0:4]=h2-field, 4:8=h1-field, 8:12=x-field
            # act of iter s writes he[:,:,s+2,0:8] = [h2_{s-2} | h1_s]
            # x_t lives at he[:, k, t+1, 8:12]; H[t]=h2_t at he[:, k, t+4, 0:4]
            he = pp.tile([128, KC, T + 4, 12], BF, tag="he")
            # decoder slots: hd[:, k, s, 0:4]=h-field (h_{s-1}), 4:8=x-field (x_s)
            hd = pp.tile([128, KC, T + 1, 8], BF, tag="hd")
            ht_all = pp.tile([128, BL, KC, 128], BF, tag="ht")  # H_b^T [t,b,k,d]
            ctxs = pp.tile([128, KC, T, BL], BF, tag="ctxs")    # ctx' [d,k,t,b]
            houts = pp.tile([128, KC, T, BL], BF, tag="houts")  # outs' [d,k,t,b]

            # ---- small loads on the SP queue, then the big W prefetch ----
            for k in range(KC):
                nc.sync.dma_start(out=u_sb[:, k, :], in_=u_d[k * 128:(k + 1) * 128, :])
                nc.sync.dma_start(out=cwt_sb[:, k, :],
                                  in_=cwt_d[k * 128:(k + 1) * 128, :])
            nc.sync.dma_start(out=si_sb[:, :], in_=si_d[:, :])
            nc.sync.dma_start(out=ti_sb[:, :], in_=ti_d[:, :])
            nc.sync.dma_start(out=b2row[:, :], in_=b2r_d[:, :])

            make_identity(nc, ident[:, :])
            nc.vector.tensor_copy(out=identb[:, :], in_=ident[:, :])
            nc.vector.memset(ones1[:, :], 1.0)
            # zero-init slots read before first writes
            nc.vector.memset(he[:, :, 0, :], 0.0)
            nc.vector.memset(he[:, :, 1, 0:8], 0.0)
            # x-fields of the two encoder tail slots are read (never used)
            # by the uniform combined idmm — keep them finite
            nc.vector.memset(he[:, :, T, 8:12], 0.0)
            nc.vector.memset(he[:, :, T + 1, 8:12], 0.0)

            with tc.tile_pool(name="psm", bufs=2, space="PSUM") as psm:
                # ---- gather embeddings; transpose encoder x to [d, t] now,
                # decoder x is transposed later (quarters fed into decoder
                # tanh-wait windows) so the encoder chain starts sooner ----
                xge = wp.tile([T, BL, D], BF, tag="xg", bufs=2)
                for b in range(BL):
                    nc.gpsimd.indirect_dma_start(
                        out=xge[:, b, :], out_offset=None, in_=een_d[:, :],
                        in_offset=bass.IndirectOffsetOnAxis(
                            ap=si_sb[:, b:b + 1], axis=0))
                xgd = wp.tile([T, BL, D], BF, tag="xg", bufs=2)
                for b in range(BL):
                    nc.gpsimd.indirect_dma_start(
                        out=xgd[:, b, :], out_offset=None, in_=ede_d[:, :],
                        in_offset=bass.IndirectOffsetOnAxis(
                            ap=ti_sb[:, b:b + 1], axis=0))
                for b in range(BL):
                    for k in range(KC):
                        tp = psm.tile([128, 128], DT, tag="tp")
                        nc.tensor.matmul(
                            out=tp[:, :],
                            lhsT=xge[:, b, k * 128:(k + 1) * 128],
                            rhs=identb[:, :], start=True, stop=True)
                        nc.vector.tensor_copy(
                            out=he[:, k, 0:T, 8 + b], in_=tp[:, :])

                # W_out prefetch after the gathers. The shared DMA fabric is
                # FCFS, so gate each chunk behind the last gather with a
                # 1-element dummy write (WAW dep) — otherwise the big W
                # transfers win the fabric and stall the RNN start by ~50us.
                for k in range(KC):
                    for q in range(4):
                        nc.vector.tensor_copy(
                            out=w_sb[0:1, k, q * 8000:q * 8000 + 1],
                            in_=xgd[0:1, 0, 0:1])
                        nc.sync.dma_start(
                            out=w_sb[:, k, q * 8000:(q + 1) * 8000],
                            in_=wot_d[k * 128:(k + 1) * 128,
                                      q * 8000:(q + 1) * 8000])

                # ---- encoder: iterations s = 0 .. T+1 ----
                # P[:, m, 0:4] = h2-part: U^T h2_{s-3} + h1_{s-2} (+ b2)
                # P[:, m, 4:8] = h1-part: U^T h1_{s-1} + x_s      (s < T)
                with tc.tile_pool(name="pse", bufs=2, space="PSUM") as pse:
                    for s in range(T + 2):
                        h1p = s < T  # compute the h1-part this iteration?
                        nccols = 8 if h1p else 4
                        # one 2KB bank (= zero region) per d-chunk m so each
                        # chunk's accumulation group starts with a matmul that
                        # covers every column it will ever touch
                        P = pse.tile([128, KC, 8], DT, tag="pe")
                        # one idmm covers every column the group will touch:
                        # [h1-carry | x] for both d-chunks (contiguous out AP)
                        nc.tensor.matmul(
                            out=P[:, :, 0:8], lhsT=identb[:, :],
                            rhs=he[:, :, s, 4:12],
                            start=True, stop=False)
                        if with_b2:
                            for m in range(KC):
                                nc.tensor.matmul(
                                    out=P[:, m, 0:4],
                                    lhsT=b2row[:, m * 128:(m + 1) * 128],
                                    rhs=ones1[:, 0:4],
                                    start=False, stop=False)
                        # critical: U-matmuls reading slot s+1
                        for m in range(KC):
                            for k in range(KC):
                                nc.tensor.matmul(
                                    out=P[:, m, 0:nccols],
                                    lhsT=u_sb[:, k, m * 128:(m + 1) * 128],
                                    rhs=he[:, k, s + 1, 0:nccols],
                                    start=False,
                                    stop=(m == KC - 1 and k == KC - 1))
                        nc.scalar.activation(
                            out=he[:, :, s + 2, 0:nccols],
                            in_=P[:, :, 0:nccols], func=AF.Tanh)
                        # H^T quarter transposes ride the encoder windows
                        if s in (34, 66, 98):
                            q = (s - 34) // 32
                            for b in range(BL):
                                for k in range(KC):
                                    tq = psm.tile([32, 128], DT, tag="tq", bufs=1)
                                    nc.tensor.matmul(
                                        out=tq[:, :],
                                        lhsT=he[:, k, 4 + 32 * q:36 + 32 * q, b],
                                        rhs=identb[:, :], start=True, stop=True)
                                    nc.vector.tensor_copy(
                                        out=ht_all[32 * q:32 * q + 32, b, k, :],
                                        in_=tq[:, :])

                # ---- mask: maddb[b, t_src] = (src==0) * -1e9 (bf16 row) ----
                mf = wp.tile([T, BL], DT, tag="mf")
                nc.vector.tensor_copy(out=mf[:, :], in_=si_sb[:, :])
                m01 = wp.tile([T, BL], DT, tag="m01")
                nc.vector.tensor_scalar(
                    out=m01[:, :], in0=mf[:, :], scalar1=0.0, scalar2=None,
                    op0=ALU.is_equal)
                mps = psm.tile([BL, T], DT, tag="mps", bufs=1)
                nc.tensor.matmul(out=mps[:, :], lhsT=m01[:, :], rhs=ident[:, :],
                                 start=True, stop=True)
                ms4 = wp.tile([BL, T], BF, tag="ms4")
                nc.vector.tensor_scalar(
                    out=ms4[:, :], in0=mps[:, :], scalar1=-1e9,
                    scalar2=None, op0=ALU.mult)
                # fold the 4 partition rows into free dim of partition 0
                nc.sync.dma_start(out=maddb[0:1, :, :], in_=ms4[:, :])

                # last H^T quarter (needs the final encoder act)
                for b in range(BL):
                    for k in range(KC):
                        tq = psm.tile([32, 128], DT, tag="tq", bufs=1)
                        nc.tensor.matmul(
                            out=tq[:, :], lhsT=he[:, k, 100:132, b],
                            rhs=identb[:, :], start=True, stop=True)
                        nc.vector.tensor_copy(
                            out=ht_all[96:128, b, k, :], in_=tq[:, :])

            # ---- decoder + blockwise attention + interleaved projection ----
            # Work items (attention sub-steps, out-projections, projection
            # groups) are fed one per decoder step into the tanh-wait windows.
            feed_hi = []   # attention + out-projection items
            feed_lo = []   # final-projection groups
            strip_tiles = {}

            in_tail = [False]

            def emit_proj_group(c, j):
                # token tile c: decoder steps [c*PBLK, (c+1)*PBLK) x BL batch
                R0 = c * PBLK
                pl = psL.tile([128, VC], DT, tag="pl")
                for k in range(KC):
                    nc.tensor.matmul(
                        out=pl[:, :],
                        lhsT=houts[:, k, R0:R0 + PBLK, :],
                        rhs=w_sb[:, k, j * VC:(j + 1) * VC],
                        start=(k == 0), stop=(k == KC - 1))
                q, r = divmod(j * VC, STRIP)
                if r == 0:
                    strip_tiles[(c, q)] = lp.tile([128, STRIP], BF, tag="lt",
                                                  name="lt")
                lt = strip_tiles[(c, q)]
                # PSUM->SBUF downcast: GPSIMD cannot read PSUM, so use DVE
                # while the tanh chain runs, and alternate DVE/ACT in the tail
                if in_tail[0] and j % 2 == 1:
                    nc.scalar.copy(out=lt[:, r:r + VC], in_=pl[:, :])
                else:
                    nc.vector.tensor_copy(out=lt[:, r:r + VC], in_=pl[:, :])
                if r + VC == STRIP:
                    nc.sync.dma_start(
                        out=out_d[c * 128:(c + 1) * 128, q * STRIP:(q + 1) * STRIP],
                        in_=lt[:, :])
                    del strip_tiles[(c, q)]

            attn_st = {}

            def emit_attn_scores(c, b):
                R = slice(1 + c * TBLK, 1 + (c + 1) * TBLK)  # decoder h slots
                S = psa.tile([TBLK, T], DT, tag="pa", name="S")
                for k in range(KC):
                    nc.tensor.matmul(
                        out=S[:, :], lhsT=hd[:, k, R, b],
                        rhs=he[:, k, 4:T + 4, b],
                        start=(k == 0), stop=False)
                nc.tensor.matmul(
                    out=S[:, :], lhsT=ones1[:, 0:TBLK],
                    rhs=maddb[0:1, b, :], start=False, stop=True)
                rmax = wp.tile([TBLK, 1], DT, tag="rmax")
                nc.vector.reduce_max(out=rmax[:, :], in_=S[:, :], axis=AX.X)
                nb = wp.tile([TBLK, 1], DT, tag="nb")
                nc.vector.tensor_scalar(
                    out=nb[:, :], in0=rmax[:, :], scalar1=-1.0 / 16.0,
                    scalar2=None, op0=ALU.mult)
                attn_st[(c, b)] = (S, nb)

            def emit_attn_softmax(c, b):
                S, nb = attn_st[(c, b)]
                exs = wp.tile([TBLK, T], DT, tag="exs")
                sums = wp.tile([TBLK, 1], DT, tag="sums")
                nc.scalar.activation(
                    out=exs[:, :], in_=S[:, :], func=AF.Exp,
                    bias=nb[:, :1], scale=1.0 / 16.0,
                    accum_out=sums[:, :1])
                rs = wp.tile([TBLK, 1], DT, tag="rs")
                nc.vector.reciprocal(out=rs[:, :], in_=sums[:, :])
                alb = wp.tile([TBLK, T], BF, tag="alb")
                nc.vector.tensor_scalar(
                    out=alb[:, :], in0=exs[:, :], scalar1=rs[:, :1],
                    scalar2=None, op0=ALU.mult)
                attn_st[(c, b)] = alb

            def emit_attn_transpose(c, b):
                alb = attn_st[(c, b)]
                pt = psa.tile([T, TBLK], DT, tag="pa", name="pt")
                nc.tensor.matmul(out=pt[:, :], lhsT=alb[:, :],
                                 rhs=identb[0:TBLK, 0:TBLK],
                                 start=True, stop=True)
                a_t = wp.tile([T, TBLK], BF, tag="a_t")
                nc.vector.tensor_copy(out=a_t[:, :], in_=pt[:, :])
                attn_st[(c, b)] = a_t

            def emit_attn_ctx(c, b):
                a_t = attn_st.pop((c, b))
                R0 = c * TBLK
                for m in range(KC):
                    pc = psa.tile([128, TBLK], DT, tag="pa", name="pc")
                    nc.tensor.matmul(
                        out=pc[:, :], lhsT=ht_all[:, b, m, :],
                        rhs=a_t[:, :], start=True, stop=True)
                    nc.vector.tensor_copy(
                        out=ctxs[:, m, R0:R0 + TBLK, b], in_=pc[:, :])

            def emit_outproj(c, m):
                # out' = h' + ctx_W @ ctx' for block c (TBLK*BL tokens)
                R = slice(1 + c * TBLK, 1 + (c + 1) * TBLK)
                R0 = c * TBLK
                po = psa.tile([128, TBLK, BL], DT, tag="pa", name="po")
                for k in range(KC):
                    nc.tensor.matmul(
                        out=po[:, :, :],
                        lhsT=cwt_sb[:, k, m * 128:(m + 1) * 128],
                        rhs=ctxs[:, k, R0:R0 + TBLK, :],
                        start=(k == 0), stop=(k == KC - 1))
                nc.vector.tensor_add(
                    out=houts[:, m, R0:R0 + TBLK, :],
                    in0=po[:, :, :], in1=hd[:, m, R, 0:4])
                if m == KC - 1:
                    for j in range(NVC):
                        feed_lo.append((emit_proj_group, (c, j)))

            with (
                tc.tile_pool(name="psd", bufs=2, space="PSUM") as psd,
                tc.tile_pool(name="psa", bufs=3, space="PSUM") as psa,
                tc.tile_pool(name="psL", bufs=3, space="PSUM") as psL,
            ):
                def emit_decx(b, k, q):
                    # transpose one 32-step quarter of decoder x into hd
                    tx = psa.tile([128, 32], DT, tag="pa", name="tx")
                    nc.tensor.matmul(
                        out=tx[:, :], lhsT=xgd[:, b, k * 128:(k + 1) * 128],
                        rhs=identb[:, 32 * q:32 * q + 32], start=True, stop=True)
                    nc.vector.tensor_copy(
                        out=hd[:, k, 32 * q:32 * q + 32, 4 + b], in_=tx[:, :])

                for b in range(BL):
                    for k in range(KC):
                        emit_decx(b, k, 0)
                for q in range(1, 4):
                    for b in range(BL):
                        for k in range(KC):
                            feed_hi.append((emit_decx, (b, k, q)))

                for s in range(T):
                    P = psd.tile([128, KC, 4], DT, tag="pd")
                    # combined x idmm covers both chunks — prefetchable
                    nc.tensor.matmul(
                        out=P[:, :, :], lhsT=identb[:, :],
                        rhs=hd[:, :, s, 4:8], start=True, stop=False)
                    for m in range(KC):  # critical h-recurrence
                        for k in range(KC):
                            rhs = (he[:, k, T + 3, 0:4] if s == 0
                                   else hd[:, k, s, 0:4])
                            nc.tensor.matmul(
                                out=P[:, m, 0:4],
                                lhsT=u_sb[:, k, m * 128:(m + 1) * 128],
                                rhs=rhs, start=False,
                                stop=(m == KC - 1 and k == KC - 1))
                    nc.scalar.activation(
                        out=hd[:, :, s + 1, 0:4], in_=P[:, :, 0:4], func=AF.Tanh)
                    if s % TBLK == TBLK - 1:
                        c = s // TBLK
                        # stage-major emission so the 4 batch pipelines
                        # interleave on each engine queue
                        for st in (emit_attn_scores, emit_attn_softmax,
                                   emit_attn_transpose, emit_attn_ctx):
                            for b in range(BL):
                                st(c, b)
                        for m in range(KC):
                            emit_outproj(c, m)
                    if feed_hi:
                        fn, args = feed_hi.pop(0)
                        fn(*args)
                    else:
                        for _ in range(NLO):
                            if feed_lo:
                                fn, args = feed_lo.pop(0)
                                fn(*args)
                # tail: drain the remaining work items
                in_tail[0] = True
                while feed_lo:
                    fn, args = feed_lo.pop(0)
                    fn(*args)
    nc.compile()
    return nc


def _prep_in_maps(U, b_enc1, b_enc2, b_dec, E_en, E_de, ctx_W, W_out_de,
                  src_en, tgt_de_in):
    f32 = np.float32
    Ub = np.ascontiguousarray(np.asarray(U, f32)).astype(NPBF)
    ctx_wt = np.ascontiguousarray(np.asarray(ctx_W, f32).T).astype(NPBF)
    w_out_t = np.ascontiguousarray(np.asarray(W_out_de, f32).T).astype(NPBF)
    # fold per-layer input biases into the embedding tables (applied to
    # every token, PAD included — matches  x + b  inside the reference tanh)
    E_en = (np.asarray(E_en, f32) + np.asarray(b_enc1, f32)[None, :]).astype(NPBF)
    E_de = (np.asarray(E_de, f32) + np.asarray(b_dec, f32)[None, :]).astype(NPBF)
    b2row = np.ascontiguousarray(np.asarray(b_enc2, f32).reshape(1, D)).astype(NPBF)
    src = np.asarray(src_en).astype(np.int32)
    tgt = np.asarray(tgt_de_in).astype(np.int32)
    in_maps = []
    for i in range(NCORES):
        b0 = i * BL
        in_maps.append({
            "u": Ub, "ctx_wt": ctx_wt, "w_out_t": w_out_t,
            "e_en": E_en, "e_de": E_de, "b2row": b2row,
            "src_idx": np.ascontiguousarray(src[:, b0:b0 + BL]),
            "tgt_idx": np.ascontiguousarray(tgt[:, b0:b0 + BL]),
        })
    return in_maps


def kernel(U, b_enc1, b_enc2, b_dec, E_en, E_de, ctx_W, W_out_de,
           src_en, tgt_de_in, _trace=False, _raw=False):
    with_b2 = bool(np.any(np.asarray(b_enc2) != 0))
    key = ("nc", with_b2)
    if key not in _CACHE:
        _CACHE[key] = _build(with_b2)
    nc = _CACHE[key]
    in_maps = _prep_in_maps(U, b_enc1, b_enc2, b_dec, E_en, E_de, ctx_W,
                            W_out_de, src_en, tgt_de_in)
    res = run_bass_kernel_spmd(nc, in_maps, list(range(NCORES)), trace=_trace)
    if _raw:
        return res
    logits = np.empty((T, B, V), np.float32)
    for i in range(NCORES):
        logits[:, i * BL:(i + 1) * BL, :] = (
            res.results[i]["out"].astype(np.float32).reshape(T, BL, V))
    if _trace:
        return logits, res
    return logits


# revision 30
# speedup vs baseline: 1.0091x; 1.0091x over previous
"""Seq2seq RNN with attention on 8 TRN2 NeuronCores.

Strategy: data-parallel over batch. B=32 -> 4 batch elements per core.

Key ideas vs a naive per-step implementation:
- The decoder's h-recurrence does not depend on attention, so the decoder
  RNN runs as a bare tanh chain and attention/context/out-projection are
  computed afterwards in large batched matmuls (per 32-step block).
- RNN steps use a slot layout in SBUF: the single fused Tanh of step s
  writes [h2_{s-2} | h1_s] directly into slot s+2; the next step's
  matmuls read the slot directly (no copies on the serial chain). Layer-2
  runs 2 steps behind layer-1 so its h1-carry matmul is off the critical
  path. Per-step critical path = 4 U-matmuls + 1 activation.
- Biases b_enc1/b_dec are folded into the embedding tables host-side;
  b_enc2 is added with a prefetched rank-1 matmul (skipped when zero).
- The full W_out.T (16MB bf16) is DMA'd into SBUF during the encoder.
  Final-projection groups (2 matmuls of [128x500] + a copy) are
  interleaved one-per-decoder-step so they execute inside the tanh-wait
  windows of the serial chain; logits are staged in 16KB/partition strips
  and stored with few large DMAs.
"""

import numpy as np

import concourse.bass as bass
import concourse.bacc as bacc
import concourse.tile as tile
from concourse import mybir
from concourse.bass_utils import run_bass_kernel_spmd
from concourse.masks import make_identity

D = 256
V = 32000
T = 128  # T_SRC == T_TGT == 128
B = 32
NCORES = 8
BL = B // NCORES  # 4 batch elements per core
KC = D // 128  # 2 d-chunks of 128
DT = mybir.dt.float32
BF = mybir.dt.bfloat16
NPBF = mybir.dt.np(BF)
AF = mybir.ActivationFunctionType
ALU = mybir.AluOpType
AX = mybir.AxisListType

TBLK = 32            # decoder timesteps per attention block
NBLK = T // TBLK     # 4 blocks
PBLK = 32            # decoder timesteps per projection token tile (128 tokens)
VC = 500             # vocab chunk per projection matmul (psum <= 2KB/part)
NVC = V // VC        # 64 chunks per token tile
STRIP = 8000         # vocab columns per logits store strip (16KB/part bf16)
NSTRIP = V // STRIP  # 4 strips per token tile

_CACHE = {}


def _build(with_b2, NLO=1):
    nc = bacc.Bacc(None)

    u_d = nc.declare_dram_parameter("u", [D, D], BF, isOutput=False)
    cwt_d = nc.declare_dram_parameter("ctx_wt", [D, D], BF, isOutput=False)
    wot_d = nc.declare_dram_parameter("w_out_t", [D, V], BF, isOutput=False)
    een_d = nc.declare_dram_parameter("e_en", [V, D], BF, isOutput=False)
    ede_d = nc.declare_dram_parameter("e_de", [V, D], BF, isOutput=False)
    b2r_d = nc.declare_dram_parameter("b2row", [1, D], BF, isOutput=False)
    si_d = nc.declare_dram_parameter("src_idx", [T, BL], mybir.dt.int32, isOutput=False)
    ti_d = nc.declare_dram_parameter("tgt_idx", [T, BL], mybir.dt.int32, isOutput=False)
    out_d = nc.declare_dram_parameter("out", [T * BL, V], BF, isOutput=True)

    with tile.TileContext(nc) as tc:
        with (
            tc.tile_pool(name="persist", bufs=1) as pp,
            tc.tile_pool(name="work", bufs=6) as wp,
            tc.tile_pool(name="strips", bufs=3) as lp,
        ):
            # ---- persistent SBUF tiles ----
            u_sb = pp.tile([128, KC, D], BF, tag="u")
            cwt_sb = pp.tile([128, KC, D], BF, tag="cwt")
            w_sb = pp.tile([128, KC, V], BF, tag="w")  # 128KB/partition
            ident = pp.tile([128, 128], DT, tag="ident")
            identb = pp.tile([128, 128], BF, tag="identb")
            ones1 = pp.tile([1, 128], BF, tag="ones1")
            b2row = pp.tile([1, D], BF, tag="b2row")
            si_sb = pp.tile([T, BL], mybir.dt.int32, tag="si")
            ti_sb = pp.tile([T, BL], mybir.dt.int32, tag="ti")
            maddb = pp.tile([1, BL, T], BF, tag="maddb")
            # encoder slots: he[:, k, s, 0:4]=h2-field, 4:8=h1-field, 8:12=x-field
            # act of iter s writes he[:,:,s+2,0:8] = [h2_{s-2} | h1_s]
            # x_t lives at he[:, k, t+1, 8:12]; H[t]=h2_t at he[:, k, t+4, 0:4]
            he = pp.tile([128, KC, T + 4, 12], BF, tag="he")
            # decoder slots: hd[:, k, s, 0:4]=h-field (h_{s-1}), 4:8=x-field (x_s)
            hd = pp.tile([128, KC, T + 1, 8], BF, tag="hd")
            ht_all = pp.tile([128, BL, KC, 128], BF, tag="ht")  # H_b^T [t,b,k,d]
            ctxs = pp.tile([128, KC, T, BL], BF, tag="ctxs")    # ctx' [d,k,t,b]
            houts = pp.tile([128, KC, T, BL], BF, tag="houts")  # outs' [d,k,t,b]

            # ---- small loads: indices first (they gate the gathers that
            # gate the whole RNN chain), weights after ----
            nc.sync.dma_start(out=si_sb[:, :], in_=si_d[:, :])
            nc.sync.dma_start(out=ti_sb[:, :], in_=ti_d[:, :])
            for k in range(KC):
                nc.sync.dma_start(out=u_sb[:, k, :], in_=u_d[k * 128:(k + 1) * 128, :])
                nc.sync.dma_start(out=cwt_sb[:, k, :],
                                  in_=cwt_d[k * 128:(k + 1) * 128, :])
            nc.sync.dma_start(out=b2row[:, :], in_=b2r_d[:, :])

            make_identity(nc, ident[:, :])
            nc.vector.tensor_copy(out=identb[:, :], in_=ident[:, :])
            nc.vector.memset(ones1[:, :], 1.0)
            # zero-init slots read before first writes
            nc.vector.memset(he[:, :, 0, :], 0.0)
            nc.vector.memset(he[:, :, 1, 0:8], 0.0)
            # x-fields of the two encoder tail slots are read (never used)
            # by the uniform combined idmm — keep them finite
            nc.vector.memset(he[:, :, T, 8:12], 0.0)
            nc.vector.memset(he[:, :, T + 1, 8:12], 0.0)

            with tc.tile_pool(name="psm", bufs=2, space="PSUM") as psm:
                # ---- gather embeddings; transpose encoder x to [d, t] now,
                # decoder x is transposed later (quarters fed into decoder
                # tanh-wait windows) so the encoder chain starts sooner ----
                xge = wp.tile([T, BL, D], BF, tag="xg", bufs=2)
                for b in range(BL):
                    nc.gpsimd.indirect_dma_start(
                        out=xge[:, b, :], out_offset=None, in_=een_d[:, :],
                        in_offset=bass.IndirectOffsetOnAxis(
                            ap=si_sb[:, b:b + 1], axis=0))
                xgd = wp.tile([T, BL, D], BF, tag="xg", bufs=2)
                for b in range(BL):
                    nc.gpsimd.indirect_dma_start(
                        out=xgd[:, b, :], out_offset=None, in_=ede_d[:, :],
                        in_offset=bass.IndirectOffsetOnAxis(
                            ap=ti_sb[:, b:b + 1], axis=0))
                for b in range(BL):
                    for k in range(KC):
                        tp = psm.tile([128, 128], DT, tag="tp")
                        nc.tensor.matmul(
                            out=tp[:, :],
                            lhsT=xge[:, b, k * 128:(k + 1) * 128],
                            rhs=identb[:, :], start=True, stop=True)
                        nc.vector.tensor_copy(
                            out=he[:, k, 0:T, 8 + b], in_=tp[:, :])

                # W_out prefetch after the gathers. The shared DMA fabric is
                # FCFS, so gate each chunk behind the last gather with a
                # 1-element dummy write (WAW dep) — otherwise the big W
                # transfers win the fabric and stall the RNN start by ~50us.
                for k in range(KC):
                    for q in range(4):
                        nc.vector.tensor_copy(
                            out=w_sb[0:1, k, q * 8000:q * 8000 + 1],
                            in_=xgd[0:1, 0, 0:1])
                        nc.sync.dma_start(
                            out=w_sb[:, k, q * 8000:(q + 1) * 8000],
                            in_=wot_d[k * 128:(k + 1) * 128,
                                      q * 8000:(q + 1) * 8000])

                # ---- encoder: iterations s = 0 .. T+1 ----
                # P[:, m, 0:4] = h2-part: U^T h2_{s-3} + h1_{s-2} (+ b2)
                # P[:, m, 4:8] = h1-part: U^T h1_{s-1} + x_s      (s < T)
                with tc.tile_pool(name="pse", bufs=2, space="PSUM") as pse:
                    for s in range(T + 2):
                        h1p = s < T  # compute the h1-part this iteration?
                        nccols = 8 if h1p else 4
                        # one 2KB bank (= zero region) per d-chunk m so each
                        # chunk's accumulation group starts with a matmul that
                        # covers every column it will ever touch
                        P = pse.tile([128, KC, 8], DT, tag="pe")
                        # one idmm covers every column the group will touch:
                        # [h1-carry | x] for both d-chunks (contiguous out AP)
                        nc.tensor.matmul(
                            out=P[:, :, 0:8], lhsT=identb[:, :],
                            rhs=he[:, :, s, 4:12],
                            start=True, stop=False)
                        if with_b2:
                            for m in range(KC):
                                nc.tensor.matmul(
                                    out=P[:, m, 0:4],
                                    lhsT=b2row[:, m * 128:(m + 1) * 128],
                                    rhs=ones1[:, 0:4],
                                    start=False, stop=False)
                        # critical: U-matmuls reading slot s+1
                        for m in range(KC):
                            for k in range(KC):
                                nc.tensor.matmul(
                                    out=P[:, m, 0:nccols],
                                    lhsT=u_sb[:, k, m * 128:(m + 1) * 128],
                                    rhs=he[:, k, s + 1, 0:nccols],
                                    start=False,
                                    stop=(m == KC - 1 and k == KC - 1))
                        nc.scalar.activation(
                            out=he[:, :, s + 2, 0:nccols],
                            in_=P[:, :, 0:nccols], func=AF.Tanh)
                        # H^T quarter transposes ride the encoder windows
                        if s in (34, 66, 98):
                            q = (s - 34) // 32
                            for b in range(BL):
                                for k in range(KC):
                                    tq = psm.tile([32, 128], DT, tag="tq", bufs=1)
                                    nc.tensor.matmul(
                                        out=tq[:, :],
                                        lhsT=he[:, k, 4 + 32 * q:36 + 32 * q, b],
                                        rhs=identb[:, :], start=True, stop=True)
                                    nc.vector.tensor_copy(
                                        out=ht_all[32 * q:32 * q + 32, b, k, :],
                                        in_=tq[:, :])

                # ---- mask: maddb[b, t_src] = (src==0) * -1e9 (bf16 row) ----
                mf = wp.tile([T, BL], DT, tag="mf")
                nc.vector.tensor_copy(out=mf[:, :], in_=si_sb[:, :])
                m01 = wp.tile([T, BL], DT, tag="m01")
                nc.vector.tensor_scalar(
                    out=m01[:, :], in0=mf[:, :], scalar1=0.0, scalar2=None,
                    op0=ALU.is_equal)
                mps = psm.tile([BL, T], DT, tag="mps", bufs=1)
                nc.tensor.matmul(out=mps[:, :], lhsT=m01[:, :], rhs=ident[:, :],
                                 start=True, stop=True)
                ms4 = wp.tile([BL, T], BF, tag="ms4")
                nc.vector.tensor_scalar(
                    out=ms4[:, :], in0=mps[:, :], scalar1=-1e9,
                    scalar2=None, op0=ALU.mult)
                # fold the 4 partition rows into free dim of partition 0
                nc.sync.dma_start(out=maddb[0:1, :, :], in_=ms4[:, :])

                # last H^T quarter (needs the final encoder act)
                for b in range(BL):
                    for k in range(KC):
                        tq = psm.tile([32, 128], DT, tag="tq", bufs=1)
                        nc.tensor.matmul(
                            out=tq[:, :], lhsT=he[:, k, 100:132, b],
                            rhs=identb[:, :], start=True, stop=True)
                        nc.vector.tensor_copy(
                            out=ht_all[96:128, b, k, :], in_=tq[:, :])

            # ---- decoder + blockwise attention + interleaved projection ----
            # Work items (attention sub-steps, out-projections, projection
            # groups) are fed one per decoder step into the tanh-wait windows.
            feed_hi = []   # attention + out-projection items
            feed_lo = []   # final-projection groups
            strip_tiles = {}

            in_tail = [False]

            def emit_proj_group(c, j):
                # token tile c: decoder steps [c*PBLK, (c+1)*PBLK) x BL batch
                R0 = c * PBLK
                pl = psL.tile([128, VC], DT, tag="pl")
                for k in range(KC):
                    nc.tensor.matmul(
                        out=pl[:, :],
                        lhsT=houts[:, k, R0:R0 + PBLK, :],
                        rhs=w_sb[:, k, j * VC:(j + 1) * VC],
                        start=(k == 0), stop=(k == KC - 1))
                q, r = divmod(j * VC, STRIP)
                stw = STRIP
                if r == 0:
                    strip_tiles[(c, q)] = lp.tile(
                        [128, STRIP], BF, tag="lt", name="lt", bufs=3)
                lt = strip_tiles[(c, q)]
                # PSUM->SBUF downcast: GPSIMD cannot read PSUM, so use DVE
                # while the tanh chain runs, and alternate DVE/ACT in the tail
                if in_tail[0] and j % 2 == 1:
                    nc.scalar.copy(out=lt[:, r:r + VC], in_=pl[:, :])
                else:
                    nc.vector.tensor_copy(out=lt[:, r:r + VC], in_=pl[:, :])
                if r + VC == stw:
                    nc.sync.dma_start(
                        out=out_d[c * 128:(c + 1) * 128, q * stw:(q + 1) * stw],
                        in_=lt[:, :])
                    del strip_tiles[(c, q)]

            attn_st = {}

            def emit_attn_scores(c, b):
                R = slice(1 + c * TBLK, 1 + (c + 1) * TBLK)  # decoder h slots
                S = psa.tile([TBLK, T], DT, tag="pa", name="S")
                for k in range(KC):
                    nc.tensor.matmul(
                        out=S[:, :], lhsT=hd[:, k, R, b],
                        rhs=he[:, k, 4:T + 4, b],
                        start=(k == 0), stop=False)
                nc.tensor.matmul(
                    out=S[:, :], lhsT=ones1[:, 0:TBLK],
                    rhs=maddb[0:1, b, :], start=False, stop=True)
                rmax = wp.tile([TBLK, 1], DT, tag="rmax")
                nc.vector.reduce_max(out=rmax[:, :], in_=S[:, :], axis=AX.X)
                nb = wp.tile([TBLK, 1], DT, tag="nb")
                nc.vector.tensor_scalar(
                    out=nb[:, :], in0=rmax[:, :], scalar1=-1.0 / 16.0,
                    scalar2=None, op0=ALU.mult)
                attn_st[(c, b)] = (S, nb)

            def emit_attn_softmax(c, b):
                S, nb = attn_st[(c, b)]
                exs = wp.tile([TBLK, T], DT, tag="exs")
                sums = wp.tile([TBLK, 1], DT, tag="sums")
                nc.scalar.activation(
                    out=exs[:, :], in_=S[:, :], func=AF.Exp,
                    bias=nb[:, :1], scale=1.0 / 16.0,
                    accum_out=sums[:, :1])
                rs = wp.tile([TBLK, 1], DT, tag="rs")
                nc.vector.reciprocal(out=rs[:, :], in_=sums[:, :])
                alb = wp.tile([TBLK, T], BF, tag="alb")
                nc.vector.tensor_scalar(
                    out=alb[:, :], in0=exs[:, :], scalar1=rs[:, :1],
                    scalar2=None, op0=ALU.mult)
                attn_st[(c, b)] = alb

            def emit_attn_transpose(c, b):
                alb = attn_st[(c, b)]
                pt = psa.tile([T, TBLK], DT, tag="pa", name="pt")
                nc.tensor.matmul(out=pt[:, :], lhsT=alb[:, :],
                                 rhs=identb[0:TBLK, 0:TBLK],
                                 start=True, stop=True)
                a_t = wp.tile([T, TBLK], BF, tag="a_t")
                nc.vector.tensor_copy(out=a_t[:, :], in_=pt[:, :])
                attn_st[(c, b)] = a_t

            def emit_attn_ctx(c, b):
                a_t = attn_st.pop((c, b))
                R0 = c * TBLK
                for m in range(KC):
                    pc = psa.tile([128, TBLK], DT, tag="pa", name="pc")
                    nc.tensor.matmul(
                        out=pc[:, :], lhsT=ht_all[:, b, m, :],
                        rhs=a_t[:, :], start=True, stop=True)
                    nc.vector.tensor_copy(
                        out=ctxs[:, m, R0:R0 + TBLK, b], in_=pc[:, :])

            def emit_outproj(c, m):
                # out' = h' + ctx_W @ ctx' for block c (TBLK*BL tokens)
                R = slice(1 + c * TBLK, 1 + (c + 1) * TBLK)
                R0 = c * TBLK
                po = psa.tile([128, TBLK, BL], DT, tag="pa", name="po")
                for k in range(KC):
                    nc.tensor.matmul(
                        out=po[:, :, :],
                        lhsT=cwt_sb[:, k, m * 128:(m + 1) * 128],
                        rhs=ctxs[:, k, R0:R0 + TBLK, :],
                        start=(k == 0), stop=(k == KC - 1))
                nc.vector.tensor_add(
                    out=houts[:, m, R0:R0 + TBLK, :],
                    in0=po[:, :, :], in1=hd[:, m, R, 0:4])
                if m == KC - 1:
                    for j in range(NVC):
                        feed_lo.append((emit_proj_group, (c, j)))

            with (
                tc.tile_pool(name="psd", bufs=2, space="PSUM") as psd,
                tc.tile_pool(name="psa", bufs=3, space="PSUM") as psa,
                tc.tile_pool(name="psL", bufs=3, space="PSUM") as psL,
            ):
                def emit_decx(b, k, q):
                    # transpose one 32-step quarter of decoder x into hd
                    tx = psa.tile([128, 32], DT, tag="pa", name="tx")
                    nc.tensor.matmul(
                        out=tx[:, :], lhsT=xgd[:, b, k * 128:(k + 1) * 128],
                        rhs=identb[:, 32 * q:32 * q + 32], start=True, stop=True)
                    nc.vector.tensor_copy(
                        out=hd[:, k, 32 * q:32 * q + 32, 4 + b], in_=tx[:, :])

                for b in range(BL):
                    for k in range(KC):
                        emit_decx(b, k, 0)
                for q in range(1, 4):
                    for b in range(BL):
                        for k in range(KC):
                            feed_hi.append((emit_decx, (b, k, q)))

                for s in range(T):
                    P = psd.tile([128, KC, 4], DT, tag="pd")
                    # combined x idmm covers both chunks — prefetchable
                    nc.tensor.matmul(
                        out=P[:, :, :], lhsT=identb[:, :],
                        rhs=hd[:, :, s, 4:8], start=True, stop=False)
                    for m in range(KC):  # critical h-recurrence
                        for k in range(KC):
                            rhs = (he[:, k, T + 3, 0:4] if s == 0
                                   else hd[:, k, s, 0:4])
                            nc.tensor.matmul(
                                out=P[:, m, 0:4],
                                lhsT=u_sb[:, k, m * 128:(m + 1) * 128],
                                rhs=rhs, start=False,
                                stop=(m == KC - 1 and k == KC - 1))
                    nc.scalar.activation(
                        out=hd[:, :, s + 1, 0:4], in_=P[:, :, 0:4], func=AF.Tanh)
                    if s % TBLK == TBLK - 1:
                        c = s // TBLK
                        # stage-major emission so the 4 batch pipelines
                        # interleave on each engine queue
                        for st in (emit_attn_scores, emit_attn_softmax,
                                   emit_attn_transpose, emit_attn_ctx):
                            for b in range(BL):
                                st(c, b)
                        for m in range(KC):
                            emit_outproj(c, m)
                    if feed_hi:
                        fn, args = feed_hi.pop(0)
                        fn(*args)
                    else:
                        for _ in range(NLO):
                            if feed_lo:
                                fn, args = feed_lo.pop(0)
                                fn(*args)
                # tail: drain the remaining work items
                in_tail[0] = True
                while feed_lo:
                    fn, args = feed_lo.pop(0)
                    fn(*args)
    nc.compile()
    return nc


def _prep_in_maps(U, b_enc1, b_enc2, b_dec, E_en, E_de, ctx_W, W_out_de,
                  src_en, tgt_de_in):
    f32 = np.float32
    Ub = np.ascontiguousarray(np.asarray(U, f32)).astype(NPBF)
    ctx_wt = np.ascontiguousarray(np.asarray(ctx_W, f32).T).astype(NPBF)
    w_out_t = np.ascontiguousarray(np.asarray(W_out_de, f32).T).astype(NPBF)
    # fold per-layer input biases into the embedding tables (applied to
    # every token, PAD included — matches  x + b  inside the reference tanh)
    E_en = (np.asarray(E_en, f32) + np.asarray(b_enc1, f32)[None, :]).astype(NPBF)
    E_de = (np.asarray(E_de, f32) + np.asarray(b_dec, f32)[None, :]).astype(NPBF)
    b2row = np.ascontiguousarray(np.asarray(b_enc2, f32).reshape(1, D)).astype(NPBF)
    src = np.asarray(src_en).astype(np.int32)
    tgt = np.asarray(tgt_de_in).astype(np.int32)
    in_maps = []
    for i in range(NCORES):
        b0 = i * BL
        in_maps.append({
            "u": Ub, "ctx_wt": ctx_wt, "w_out_t": w_out_t,
            "e_en": E_en, "e_de": E_de, "b2row": b2row,
            "src_idx": np.ascontiguousarray(src[:, b0:b0 + BL]),
            "tgt_idx": np.ascontiguousarray(tgt[:, b0:b0 + BL]),
        })
    return in_maps


def kernel(U, b_enc1, b_enc2, b_dec, E_en, E_de, ctx_W, W_out_de,
           src_en, tgt_de_in, _trace=False, _raw=False):
    with_b2 = bool(np.any(np.asarray(b_enc2) != 0))
    key = ("nc", with_b2)
    if key not in _CACHE:
        _CACHE[key] = _build(with_b2)
    nc = _CACHE[key]
    in_maps = _prep_in_maps(U, b_enc1, b_enc2, b_dec, E_en, E_de, ctx_W,
                            W_out_de, src_en, tgt_de_in)
    res = run_bass_kernel_spmd(nc, in_maps, list(range(NCORES)), trace=_trace)
    if _raw:
        return res
    logits = np.empty((T, B, V), np.float32)
    for i in range(NCORES):
        logits[:, i * BL:(i + 1) * BL, :] = (
            res.results[i]["out"].astype(np.float32).reshape(T, BL, V))
    if _trace:
        return logits, res
    return logits


# revision 31
# speedup vs baseline: 1.0276x; 1.0183x over previous
"""Seq2seq RNN with attention on 8 TRN2 NeuronCores.

Strategy: data-parallel over batch. B=32 -> 4 batch elements per core.

Key ideas vs a naive per-step implementation:
- The decoder's h-recurrence does not depend on attention, so the decoder
  RNN runs as a bare tanh chain and attention/context/out-projection are
  computed afterwards in large batched matmuls (per 32-step block).
- RNN steps use a slot layout in SBUF: the single fused Tanh of step s
  writes [h2_{s-2} | h1_s] directly into slot s+2; the next step's
  matmuls read the slot directly (no copies on the serial chain). Layer-2
  runs 2 steps behind layer-1 so its h1-carry matmul is off the critical
  path. Per-step critical path = 4 U-matmuls + 1 activation.
- Biases b_enc1/b_dec are folded into the embedding tables host-side;
  b_enc2 is added with a prefetched rank-1 matmul (skipped when zero).
- The full W_out.T (16MB bf16) is DMA'd into SBUF during the encoder.
  Final-projection groups (2 matmuls of [128x500] + a copy) are
  interleaved one-per-decoder-step so they execute inside the tanh-wait
  windows of the serial chain; logits are staged in 16KB/partition strips
  and stored with few large DMAs.
"""

import numpy as np

import concourse.bass as bass
import concourse.bacc as bacc
import concourse.tile as tile
from concourse import mybir
from concourse.bass_utils import run_bass_kernel_spmd
from concourse.masks import make_identity

D = 256
V = 32000
T = 128  # T_SRC == T_TGT == 128
B = 32
NCORES = 8
BL = B // NCORES  # 4 batch elements per core
KC = D // 128  # 2 d-chunks of 128
DT = mybir.dt.float32
BF = mybir.dt.bfloat16
NPBF = mybir.dt.np(BF)
AF = mybir.ActivationFunctionType
ALU = mybir.AluOpType
AX = mybir.AxisListType

TBLK = 32            # decoder timesteps per attention block
NBLK = T // TBLK     # 4 blocks
PBLK = 32            # decoder timesteps per projection token tile (128 tokens)
VC = 500             # vocab chunk per projection matmul (psum <= 2KB/part)
NVC = V // VC        # 64 chunks per token tile
STRIP = 8000         # vocab columns per logits store strip (16KB/part bf16)
NSTRIP = V // STRIP  # 4 strips per token tile

_CACHE = {}


def _build(with_b2, NLO=1):
    nc = bacc.Bacc(None)

    u_d = nc.declare_dram_parameter("u", [D, D], BF, isOutput=False)
    cwt_d = nc.declare_dram_parameter("ctx_wt", [D, D], BF, isOutput=False)
    wot_d = nc.declare_dram_parameter("w_out_t", [D, V], BF, isOutput=False)
    een_d = nc.declare_dram_parameter("e_en", [V, D], BF, isOutput=False)
    ede_d = nc.declare_dram_parameter("e_de", [V, D], BF, isOutput=False)
    b2r_d = nc.declare_dram_parameter("b2row", [1, D], BF, isOutput=False)
    si_d = nc.declare_dram_parameter("src_idx", [T, BL], mybir.dt.int32, isOutput=False)
    ti_d = nc.declare_dram_parameter("tgt_idx", [T, BL], mybir.dt.int32, isOutput=False)
    out_d = nc.declare_dram_parameter("out", [T * BL, V], BF, isOutput=True)

    with tile.TileContext(nc) as tc:
        with (
            tc.tile_pool(name="persist", bufs=1) as pp,
            tc.tile_pool(name="work", bufs=6) as wp,
            tc.tile_pool(name="strips", bufs=3) as lp,
        ):
            # ---- persistent SBUF tiles ----
            u_sb = pp.tile([128, KC, D], BF, tag="u")
            cwt_sb = pp.tile([128, KC, D], BF, tag="cwt")
            w_sb = pp.tile([128, KC, V], BF, tag="w")  # 128KB/partition
            ident = pp.tile([128, 128], DT, tag="ident")
            identb = pp.tile([128, 128], BF, tag="identb")
            ones1 = pp.tile([1, 128], BF, tag="ones1")
            b2row = pp.tile([1, D], BF, tag="b2row")
            si_sb = pp.tile([T, BL], mybir.dt.int32, tag="si")
            ti_sb = pp.tile([T, BL], mybir.dt.int32, tag="ti")
            maddb = pp.tile([1, BL, T], BF, tag="maddb")
            # encoder slots: he[:, k, s, 0:4]=h2-field, 4:8=h1-field, 8:12=x-field
            # act of iter s writes he[:,:,s+2,0:8] = [h2_{s-2} | h1_s]
            # x_t lives at he[:, k, t+1, 8:12]; H[t]=h2_t at he[:, k, t+4, 0:4]
            he = pp.tile([128, KC, T + 4, 12], BF, tag="he")
            # decoder slots: hd[:, k, s, 0:4]=h-field (h_{s-1}), 4:8=x-field (x_s)
            hd = pp.tile([128, KC, T + 1, 8], BF, tag="hd")
            ht_all = pp.tile([128, BL, KC, 128], BF, tag="ht")  # H_b^T [t,b,k,d]
            ctxs = pp.tile([128, KC, T, BL], BF, tag="ctxs")    # ctx' [d,k,t,b]
            houts = pp.tile([128, KC, T, BL], BF, tag="houts")  # outs' [d,k,t,b]

            # ---- small loads: indices first (they gate the gathers that
            # gate the whole RNN chain), weights after ----
            nc.sync.dma_start(out=si_sb[:, :], in_=si_d[:, :])
            nc.sync.dma_start(out=ti_sb[:, :], in_=ti_d[:, :])
            for k in range(KC):
                nc.sync.dma_start(out=u_sb[:, k, :], in_=u_d[k * 128:(k + 1) * 128, :])
                nc.sync.dma_start(out=cwt_sb[:, k, :],
                                  in_=cwt_d[k * 128:(k + 1) * 128, :])
            nc.sync.dma_start(out=b2row[:, :], in_=b2r_d[:, :])

            make_identity(nc, ident[:, :])
            nc.vector.tensor_copy(out=identb[:, :], in_=ident[:, :])
            nc.vector.memset(ones1[:, :], 1.0)
            # zero-init slots read before first writes
            nc.vector.memset(he[:, :, 0, :], 0.0)
            nc.vector.memset(he[:, :, 1, 0:8], 0.0)
            # x-fields of the two encoder tail slots are read (never used)
            # by the uniform combined idmm — keep them finite
            nc.vector.memset(he[:, :, T, 8:12], 0.0)
            nc.vector.memset(he[:, :, T + 1, 8:12], 0.0)

            with tc.tile_pool(name="psm", bufs=2, space="PSUM") as psm:
                # ---- gather embeddings; transpose encoder x to [d, t] now,
                # decoder x is transposed later (quarters fed into decoder
                # tanh-wait windows) so the encoder chain starts sooner ----
                xge = wp.tile([T, BL, D], BF, tag="xg", bufs=2)
                for b in range(BL):
                    nc.gpsimd.indirect_dma_start(
                        out=xge[:, b, :], out_offset=None, in_=een_d[:, :],
                        in_offset=bass.IndirectOffsetOnAxis(
                            ap=si_sb[:, b:b + 1], axis=0))
                xgd = wp.tile([T, BL, D], BF, tag="xg", bufs=2)
                for b in range(BL):
                    nc.gpsimd.indirect_dma_start(
                        out=xgd[:, b, :], out_offset=None, in_=ede_d[:, :],
                        in_offset=bass.IndirectOffsetOnAxis(
                            ap=ti_sb[:, b:b + 1], axis=0))
                for b in range(BL):
                    for k in range(KC):
                        tp = psm.tile([128, 128], DT, tag="tp")
                        nc.tensor.matmul(
                            out=tp[:, :],
                            lhsT=xge[:, b, k * 128:(k + 1) * 128],
                            rhs=identb[:, :], start=True, stop=True)
                        nc.vector.tensor_copy(
                            out=he[:, k, 0:T, 8 + b], in_=tp[:, :])

                # W_out prefetch after the gathers. The shared DMA fabric is
                # FCFS, so gate each chunk behind the last gather with a
                # 1-element dummy write (WAW dep) — otherwise the big W
                # transfers win the fabric and stall the RNN start by ~50us.
                for k in range(KC):
                    for q in range(4):
                        nc.vector.tensor_copy(
                            out=w_sb[0:1, k, q * 8000:q * 8000 + 1],
                            in_=xgd[0:1, 0, 0:1])
                        nc.sync.dma_start(
                            out=w_sb[:, k, q * 8000:(q + 1) * 8000],
                            in_=wot_d[k * 128:(k + 1) * 128,
                                      q * 8000:(q + 1) * 8000])

                # ---- encoder: iterations s = 0 .. T+1 ----
                # P[:, m, 0:4] = h2-part: U^T h2_{s-3} + h1_{s-2} (+ b2)
                # P[:, m, 4:8] = h1-part: U^T h1_{s-1} + x_s      (s < T)
                with tc.tile_pool(name="pse", bufs=2, space="PSUM") as pse:
                    for s in range(T + 2):
                        h1p = s < T  # compute the h1-part this iteration?
                        nccols = 8 if h1p else 4
                        # one 2KB bank (= zero region) per d-chunk m so each
                        # chunk's accumulation group starts with a matmul that
                        # covers every column it will ever touch
                        P = pse.tile([128, KC, 8], DT, tag="pe")
                        # one idmm covers every column the group will touch:
                        # [h1-carry | x] for both d-chunks (contiguous out AP)
                        nc.tensor.matmul(
                            out=P[:, :, 0:8], lhsT=identb[:, :],
                            rhs=he[:, :, s, 4:12],
                            start=True, stop=False)
                        if with_b2:
                            for m in range(KC):
                                nc.tensor.matmul(
                                    out=P[:, m, 0:4],
                                    lhsT=b2row[:, m * 128:(m + 1) * 128],
                                    rhs=ones1[:, 0:4],
                                    start=False, stop=False)
                        # critical: U-matmuls reading slot s+1
                        for m in range(KC):
                            for k in range(KC):
                                nc.tensor.matmul(
                                    out=P[:, m, 0:nccols],
                                    lhsT=u_sb[:, k, m * 128:(m + 1) * 128],
                                    rhs=he[:, k, s + 1, 0:nccols],
                                    start=False,
                                    stop=(m == KC - 1 and k == KC - 1))
                        nc.scalar.activation(
                            out=he[:, :, s + 2, 0:nccols],
                            in_=P[:, :, 0:nccols], func=AF.Tanh)
                        # H^T quarter transposes ride the encoder windows
                        if s in (34, 66, 98):
                            q = (s - 34) // 32
                            for b in range(BL):
                                for k in range(KC):
                                    tq = psm.tile([32, 128], DT, tag="tq", bufs=1)
                                    nc.tensor.matmul(
                                        out=tq[:, :],
                                        lhsT=he[:, k, 4 + 32 * q:36 + 32 * q, b],
                                        rhs=identb[:, :], start=True, stop=True)
                                    nc.vector.tensor_copy(
                                        out=ht_all[32 * q:32 * q + 32, b, k, :],
                                        in_=tq[:, :])


            # ---- decoder + blockwise attention + interleaved projection ----
            # Work items (attention sub-steps, out-projections, projection
            # groups) are fed one per decoder step into the tanh-wait windows.
            feed_hi = []   # attention + out-projection items
            feed_lo = []   # final-projection groups
            strip_tiles = {}

            in_tail = [False]

            def emit_proj_group(c, j):
                # token tile c: decoder steps [c*PBLK, (c+1)*PBLK) x BL batch
                R0 = c * PBLK
                pl = psL.tile([128, VC], DT, tag="pl")
                for k in range(KC):
                    nc.tensor.matmul(
                        out=pl[:, :],
                        lhsT=houts[:, k, R0:R0 + PBLK, :],
                        rhs=w_sb[:, k, j * VC:(j + 1) * VC],
                        start=(k == 0), stop=(k == KC - 1))
                q, r = divmod(j * VC, STRIP)
                stw = STRIP
                if r == 0:
                    strip_tiles[(c, q)] = lp.tile(
                        [128, STRIP], BF, tag="lt", name="lt", bufs=3)
                lt = strip_tiles[(c, q)]
                # PSUM->SBUF downcast: GPSIMD cannot read PSUM, so use DVE
                # while the tanh chain runs, and alternate DVE/ACT in the tail
                if in_tail[0] and j % 2 == 1:
                    nc.scalar.copy(out=lt[:, r:r + VC], in_=pl[:, :])
                else:
                    nc.vector.tensor_copy(out=lt[:, r:r + VC], in_=pl[:, :])
                if r + VC == stw:
                    nc.sync.dma_start(
                        out=out_d[c * 128:(c + 1) * 128, q * stw:(q + 1) * stw],
                        in_=lt[:, :])
                    del strip_tiles[(c, q)]

            attn_st = {}

            def emit_attn_scores(c, b):
                R = slice(1 + c * TBLK, 1 + (c + 1) * TBLK)  # decoder h slots
                S = psa.tile([TBLK, T], DT, tag="pa", name="S")
                for k in range(KC):
                    nc.tensor.matmul(
                        out=S[:, :], lhsT=hd[:, k, R, b],
                        rhs=he[:, k, 4:T + 4, b],
                        start=(k == 0), stop=False)
                nc.tensor.matmul(
                    out=S[:, :], lhsT=ones1[:, 0:TBLK],
                    rhs=maddb[0:1, b, :], start=False, stop=True)
                rmax = wp.tile([TBLK, 1], DT, tag="rmax")
                nc.vector.reduce_max(out=rmax[:, :], in_=S[:, :], axis=AX.X)
                nb = wp.tile([TBLK, 1], DT, tag="nb")
                nc.vector.tensor_scalar(
                    out=nb[:, :], in0=rmax[:, :], scalar1=-1.0 / 16.0,
                    scalar2=None, op0=ALU.mult)
                attn_st[(c, b)] = (S, nb)

            def emit_attn_softmax(c, b):
                S, nb = attn_st[(c, b)]
                exs = wp.tile([TBLK, T], DT, tag="exs")
                sums = wp.tile([TBLK, 1], DT, tag="sums")
                nc.scalar.activation(
                    out=exs[:, :], in_=S[:, :], func=AF.Exp,
                    bias=nb[:, :1], scale=1.0 / 16.0,
                    accum_out=sums[:, :1])
                rs = wp.tile([TBLK, 1], DT, tag="rs")
                nc.vector.reciprocal(out=rs[:, :], in_=sums[:, :])
                alb = wp.tile([TBLK, T], BF, tag="alb")
                nc.vector.tensor_scalar(
                    out=alb[:, :], in0=exs[:, :], scalar1=rs[:, :1],
                    scalar2=None, op0=ALU.mult)
                attn_st[(c, b)] = alb

            def emit_attn_transpose(c, b):
                alb = attn_st[(c, b)]
                pt = psa.tile([T, TBLK], DT, tag="pa", name="pt")
                nc.tensor.matmul(out=pt[:, :], lhsT=alb[:, :],
                                 rhs=identb[0:TBLK, 0:TBLK],
                                 start=True, stop=True)
                a_t = wp.tile([T, TBLK], BF, tag="a_t")
                nc.vector.tensor_copy(out=a_t[:, :], in_=pt[:, :])
                attn_st[(c, b)] = a_t

            def emit_attn_ctx(c, b):
                a_t = attn_st.pop((c, b))
                R0 = c * TBLK
                for m in range(KC):
                    pc = psa.tile([128, TBLK], DT, tag="pa", name="pc")
                    nc.tensor.matmul(
                        out=pc[:, :], lhsT=ht_all[:, b, m, :],
                        rhs=a_t[:, :], start=True, stop=True)
                    nc.vector.tensor_copy(
                        out=ctxs[:, m, R0:R0 + TBLK, b], in_=pc[:, :])

            def emit_outproj(c, m):
                # out' = h' + ctx_W @ ctx' for block c (TBLK*BL tokens)
                R = slice(1 + c * TBLK, 1 + (c + 1) * TBLK)
                R0 = c * TBLK
                po = psa.tile([128, TBLK, BL], DT, tag="pa", name="po")
                for k in range(KC):
                    nc.tensor.matmul(
                        out=po[:, :, :],
                        lhsT=cwt_sb[:, k, m * 128:(m + 1) * 128],
                        rhs=ctxs[:, k, R0:R0 + TBLK, :],
                        start=(k == 0), stop=(k == KC - 1))
                nc.vector.tensor_add(
                    out=houts[:, m, R0:R0 + TBLK, :],
                    in0=po[:, :, :], in1=hd[:, m, R, 0:4])
                if m == KC - 1:
                    for j in range(NVC):
                        feed_lo.append((emit_proj_group, (c, j)))

            with (
                tc.tile_pool(name="psd", bufs=2, space="PSUM") as psd,
                tc.tile_pool(name="psa", bufs=3, space="PSUM") as psa,
                tc.tile_pool(name="psL", bufs=3, space="PSUM") as psL,
            ):
                def emit_decx(b, k, q):
                    # transpose one 32-step quarter of decoder x into hd
                    tx = psa.tile([128, 32], DT, tag="pa", name="tx")
                    nc.tensor.matmul(
                        out=tx[:, :], lhsT=xgd[:, b, k * 128:(k + 1) * 128],
                        rhs=identb[:, 32 * q:32 * q + 32], start=True, stop=True)
                    nc.vector.tensor_copy(
                        out=hd[:, k, 32 * q:32 * q + 32, 4 + b], in_=tx[:, :])

                def emit_mask():
                    # maddb[b, t_src] = (src==0) * -1e9 (bf16 row)
                    mf = wp.tile([T, BL], DT, tag="mf")
                    nc.vector.tensor_copy(out=mf[:, :], in_=si_sb[:, :])
                    m01 = wp.tile([T, BL], DT, tag="m01")
                    nc.vector.tensor_scalar(
                        out=m01[:, :], in0=mf[:, :], scalar1=0.0, scalar2=None,
                        op0=ALU.is_equal)
                    mps = psa.tile([BL, T], DT, tag="pa", name="mps")
                    nc.tensor.matmul(out=mps[:, :], lhsT=m01[:, :],
                                     rhs=ident[:, :], start=True, stop=True)
                    ms4 = wp.tile([BL, T], BF, tag="ms4")
                    nc.vector.tensor_scalar(
                        out=ms4[:, :], in0=mps[:, :], scalar1=-1e9,
                        scalar2=None, op0=ALU.mult)
                    nc.sync.dma_start(out=maddb[0:1, :, :], in_=ms4[:, :])

                def emit_htq3(b, k):
                    # last H^T quarter (needs the final encoder act)
                    tq = psa.tile([32, 128], DT, tag="pa", name="tq")
                    nc.tensor.matmul(
                        out=tq[:, :], lhsT=he[:, k, 100:132, b],
                        rhs=identb[:, :], start=True, stop=True)
                    nc.vector.tensor_copy(
                        out=ht_all[96:128, b, k, :], in_=tq[:, :])

                for b in range(BL):
                    for k in range(KC):
                        emit_decx(b, k, 0)
                feed_hi.append((emit_mask, ()))
                for b in range(BL):
                    for k in range(KC):
                        feed_hi.append((emit_htq3, (b, k)))
                for q in range(1, 4):
                    for b in range(BL):
                        for k in range(KC):
                            feed_hi.append((emit_decx, (b, k, q)))

                for s in range(T):
                    P = psd.tile([128, KC, 4], DT, tag="pd")
                    # combined x idmm covers both chunks — prefetchable
                    nc.tensor.matmul(
                        out=P[:, :, :], lhsT=identb[:, :],
                        rhs=hd[:, :, s, 4:8], start=True, stop=False)
                    for m in range(KC):  # critical h-recurrence
                        for k in range(KC):
                            rhs = (he[:, k, T + 3, 0:4] if s == 0
                                   else hd[:, k, s, 0:4])
                            nc.tensor.matmul(
                                out=P[:, m, 0:4],
                                lhsT=u_sb[:, k, m * 128:(m + 1) * 128],
                                rhs=rhs, start=False,
                                stop=(m == KC - 1 and k == KC - 1))
                    nc.scalar.activation(
                        out=hd[:, :, s + 1, 0:4], in_=P[:, :, 0:4], func=AF.Tanh)
                    if s % TBLK == TBLK - 1:
                        c = s // TBLK
                        # stage-major emission so the 4 batch pipelines
                        # interleave on each engine queue
                        for st in (emit_attn_scores, emit_attn_softmax,
                                   emit_attn_transpose, emit_attn_ctx):
                            for b in range(BL):
                                st(c, b)
                        for m in range(KC):
                            emit_outproj(c, m)
                    if feed_hi:
                        fn, args = feed_hi.pop(0)
                        fn(*args)
                    else:
                        for _ in range(NLO):
                            if feed_lo:
                                fn, args = feed_lo.pop(0)
                                fn(*args)
                # tail: drain the remaining work items
                in_tail[0] = True
                while feed_lo:
                    fn, args = feed_lo.pop(0)
                    fn(*args)
    nc.compile()
    return nc


def _prep_in_maps(U, b_enc1, b_enc2, b_dec, E_en, E_de, ctx_W, W_out_de,
                  src_en, tgt_de_in):
    f32 = np.float32
    Ub = np.ascontiguousarray(np.asarray(U, f32)).astype(NPBF)
    ctx_wt = np.ascontiguousarray(np.asarray(ctx_W, f32).T).astype(NPBF)
    w_out_t = np.ascontiguousarray(np.asarray(W_out_de, f32).T).astype(NPBF)
    # fold per-layer input biases into the embedding tables (applied to
    # every token, PAD included — matches  x + b  inside the reference tanh)
    E_en = (np.asarray(E_en, f32) + np.asarray(b_enc1, f32)[None, :]).astype(NPBF)
    E_de = (np.asarray(E_de, f32) + np.asarray(b_dec, f32)[None, :]).astype(NPBF)
    b2row = np.ascontiguousarray(np.asarray(b_enc2, f32).reshape(1, D)).astype(NPBF)
    src = np.asarray(src_en).astype(np.int32)
    tgt = np.asarray(tgt_de_in).astype(np.int32)
    in_maps = []
    for i in range(NCORES):
        b0 = i * BL
        in_maps.append({
            "u": Ub, "ctx_wt": ctx_wt, "w_out_t": w_out_t,
            "e_en": E_en, "e_de": E_de, "b2row": b2row,
            "src_idx": np.ascontiguousarray(src[:, b0:b0 + BL]),
            "tgt_idx": np.ascontiguousarray(tgt[:, b0:b0 + BL]),
        })
    return in_maps


def kernel(U, b_enc1, b_enc2, b_dec, E_en, E_de, ctx_W, W_out_de,
           src_en, tgt_de_in, _trace=False, _raw=False):
    with_b2 = bool(np.any(np.asarray(b_enc2) != 0))
    key = ("nc", with_b2)
    if key not in _CACHE:
        _CACHE[key] = _build(with_b2)
    nc = _CACHE[key]
    in_maps = _prep_in_maps(U, b_enc1, b_enc2, b_dec, E_en, E_de, ctx_W,
                            W_out_de, src_en, tgt_de_in)
    res = run_bass_kernel_spmd(nc, in_maps, list(range(NCORES)), trace=_trace)
    if _raw:
        return res
    logits = np.empty((T, B, V), np.float32)
    for i in range(NCORES):
        logits[:, i * BL:(i + 1) * BL, :] = (
            res.results[i]["out"].astype(np.float32).reshape(T, BL, V))
    if _trace:
        return logits, res
    return logits


# revision 33
# speedup vs baseline: 1.0478x; 1.0197x over previous
"""Seq2seq RNN with attention on 8 TRN2 NeuronCores.

Strategy: data-parallel over batch. B=32 -> 4 batch elements per core.

Key ideas vs a naive per-step implementation:
- The decoder's h-recurrence does not depend on attention, so the decoder
  RNN runs as a bare tanh chain and attention/context/out-projection are
  computed afterwards in large batched matmuls (per 32-step block).
- RNN steps use a slot layout in SBUF: the single fused Tanh of step s
  writes [h2_{s-2} | h1_s] directly into slot s+2; the next step's
  matmuls read the slot directly (no copies on the serial chain). Layer-2
  runs 2 steps behind layer-1 so its h1-carry matmul is off the critical
  path. Per-step critical path = 4 U-matmuls + 1 activation.
- Biases b_enc1/b_dec are folded into the embedding tables host-side;
  b_enc2 is added with a prefetched rank-1 matmul (skipped when zero).
- The full W_out.T (16MB bf16) is DMA'd into SBUF during the encoder.
  Final-projection groups (2 matmuls of [128x500] + a copy) are
  interleaved one-per-decoder-step so they execute inside the tanh-wait
  windows of the serial chain; logits are staged in 16KB/partition strips
  and stored with few large DMAs.
"""

import numpy as np

import concourse.bass as bass
import concourse.bacc as bacc
import concourse.tile as tile
from concourse import mybir
from concourse.bass_utils import run_bass_kernel_spmd
from concourse.masks import make_identity

D = 256
V = 32000
T = 128  # T_SRC == T_TGT == 128
B = 32
NCORES = 8
BL = B // NCORES  # 4 batch elements per core
KC = D // 128  # 2 d-chunks of 128
DT = mybir.dt.float32
BF = mybir.dt.bfloat16
NPBF = mybir.dt.np(BF)
AF = mybir.ActivationFunctionType
ALU = mybir.AluOpType
AX = mybir.AxisListType

TBLK = 32            # decoder timesteps per attention block
NBLK = T // TBLK     # 4 blocks
PBLK = 32            # decoder timesteps per projection token tile (128 tokens)
VC = 500             # vocab chunk per projection matmul (psum <= 2KB/part)
NVC = V // VC        # 64 chunks per token tile
STRIP = 8000         # vocab columns per logits store strip (16KB/part bf16)
NSTRIP = V // STRIP  # 4 strips per token tile

_CACHE = {}


def _build(with_b2, NLO=1):
    nc = bacc.Bacc(None)

    u_d = nc.declare_dram_parameter("u", [D, D], BF, isOutput=False)
    cwt_d = nc.declare_dram_parameter("ctx_wt", [D, D], BF, isOutput=False)
    wot_d = nc.declare_dram_parameter("w_out_t", [D, V], BF, isOutput=False)
    een_d = nc.declare_dram_parameter("e_en", [V, D], BF, isOutput=False)
    ede_d = nc.declare_dram_parameter("e_de", [V, D], BF, isOutput=False)
    b2r_d = nc.declare_dram_parameter("b2row", [1, D], BF, isOutput=False)
    si_d = nc.declare_dram_parameter("src_idx", [T, BL], mybir.dt.int32, isOutput=False)
    ti_d = nc.declare_dram_parameter("tgt_idx", [T, BL], mybir.dt.int32, isOutput=False)
    out_d = nc.declare_dram_parameter("out", [T * BL, V], BF, isOutput=True)

    with tile.TileContext(nc) as tc:
        with (
            tc.tile_pool(name="persist", bufs=1) as pp,
            tc.tile_pool(name="work", bufs=6) as wp,
            tc.tile_pool(name="strips", bufs=3) as lp,
        ):
            # ---- persistent SBUF tiles ----
            u_sb = pp.tile([128, KC, D], BF, tag="u")
            cwt_sb = pp.tile([128, KC, D], BF, tag="cwt")
            w_sb = pp.tile([128, KC, V], BF, tag="w")  # 128KB/partition
            ident = pp.tile([128, 128], DT, tag="ident")
            identb = pp.tile([128, 128], BF, tag="identb")
            ones1 = pp.tile([1, 128], BF, tag="ones1")
            b2row = pp.tile([1, D], BF, tag="b2row")
            si_sb = pp.tile([T, BL], mybir.dt.int32, tag="si")
            ti_sb = pp.tile([T, BL], mybir.dt.int32, tag="ti")
            maddb = pp.tile([1, BL, T], BF, tag="maddb")
            # encoder slots: he[:, k, s, 0:4]=h2-field, 4:8=h1-field, 8:12=x-field
            # act of iter s writes he[:,:,s+2,0:8] = [h2_{s-2} | h1_s]
            # x_t lives at he[:, k, t+1, 8:12]; H[t]=h2_t at he[:, k, t+4, 0:4]
            he = pp.tile([128, KC, T + 4, 12], BF, tag="he")
            # decoder slots: hd[:, k, s, 0:4]=h-field (h_{s-1}), 4:8=x-field (x_s)
            hd = pp.tile([128, KC, T + 1, 8], BF, tag="hd")
            ht_all = pp.tile([128, BL, KC, 128], BF, tag="ht")  # H_b^T [t,b,k,d]
            ctxs = pp.tile([128, KC, T, BL], BF, tag="ctxs")    # ctx' [d,k,t,b]
            houts = pp.tile([128, KC, T, BL], BF, tag="houts")  # outs' [d,k,t,b]

            # ---- small loads: indices first (they gate the gathers that
            # gate the whole RNN chain), weights after ----
            nc.sync.dma_start(out=si_sb[:, :], in_=si_d[:, :])
            nc.sync.dma_start(out=ti_sb[:, :], in_=ti_d[:, :])
            for k in range(KC):
                nc.sync.dma_start(out=u_sb[:, k, :], in_=u_d[k * 128:(k + 1) * 128, :])
                nc.sync.dma_start(out=cwt_sb[:, k, :],
                                  in_=cwt_d[k * 128:(k + 1) * 128, :])
            nc.sync.dma_start(out=b2row[:, :], in_=b2r_d[:, :])

            make_identity(nc, ident[:, :])
            nc.vector.tensor_copy(out=identb[:, :], in_=ident[:, :])
            nc.vector.memset(ones1[:, :], 1.0)
            # zero-init slots read before first writes
            nc.vector.memset(he[:, :, 0, :], 0.0)
            nc.vector.memset(he[:, :, 1, 0:8], 0.0)
            # x-fields of the two encoder tail slots are read (never used)
            # by the uniform combined idmm — keep them finite
            nc.vector.memset(he[:, :, T, 8:12], 0.0)
            nc.vector.memset(he[:, :, T + 1, 8:12], 0.0)

            with tc.tile_pool(name="psm", bufs=2, space="PSUM") as psm:
                # ---- gather embeddings; transpose encoder x to [d, t] now,
                # decoder x is transposed later (quarters fed into decoder
                # tanh-wait windows) so the encoder chain starts sooner ----
                xge = wp.tile([T, BL, D], BF, tag="xg", bufs=2)
                for b in range(BL):
                    nc.gpsimd.indirect_dma_start(
                        out=xge[:, b, :], out_offset=None, in_=een_d[:, :],
                        in_offset=bass.IndirectOffsetOnAxis(
                            ap=si_sb[:, b:b + 1], axis=0))
                xgd = wp.tile([T, BL, D], BF, tag="xg", bufs=2)
                for b in range(BL):
                    nc.gpsimd.indirect_dma_start(
                        out=xgd[:, b, :], out_offset=None, in_=ede_d[:, :],
                        in_offset=bass.IndirectOffsetOnAxis(
                            ap=ti_sb[:, b:b + 1], axis=0))
                for b in range(BL):
                    for k in range(KC):
                        tp = psm.tile([128, 128], DT, tag="tp")
                        nc.tensor.matmul(
                            out=tp[:, :],
                            lhsT=xge[:, b, k * 128:(k + 1) * 128],
                            rhs=identb[:, :], start=True, stop=True)
                        nc.vector.tensor_copy(
                            out=he[:, k, 0:T, 8 + b], in_=tp[:, :])

                # W_out prefetch after the gathers. The shared DMA fabric is
                # FCFS, so gate each chunk behind the last gather with a
                # 1-element dummy write (WAW dep) — otherwise the big W
                # transfers win the fabric and stall the RNN start by ~50us.
                for k in range(KC):
                    for q in range(4):
                        nc.vector.tensor_copy(
                            out=w_sb[0:1, k, q * 8000:q * 8000 + 1],
                            in_=xgd[0:1, 0, 0:1])
                        nc.sync.dma_start(
                            out=w_sb[:, k, q * 8000:(q + 1) * 8000],
                            in_=wot_d[k * 128:(k + 1) * 128,
                                      q * 8000:(q + 1) * 8000])

                # ---- encoder: iterations s = 0 .. T+1 ----
                # P[:, m, 0:4] = h2-part: U^T h2_{s-3} + h1_{s-2} (+ b2)
                # P[:, m, 4:8] = h1-part: U^T h1_{s-1} + x_s      (s < T)
                with tc.tile_pool(name="pse", bufs=2, space="PSUM") as pse:
                    for s in range(T + 2):
                        h1p = s < T  # compute the h1-part this iteration?
                        nccols = 8 if h1p else 4
                        # one 2KB bank (= zero region) per d-chunk m so each
                        # chunk's accumulation group starts with a matmul that
                        # covers every column it will ever touch
                        P = pse.tile([128, KC, 8], DT, tag="pe")
                        # one idmm covers every column the group will touch:
                        # [h1-carry | x] for both d-chunks (contiguous out AP)
                        nc.tensor.matmul(
                            out=P[:, :, 0:8], lhsT=identb[:, :],
                            rhs=he[:, :, s, 4:12],
                            start=True, stop=False)
                        if with_b2:
                            for m in range(KC):
                                nc.tensor.matmul(
                                    out=P[:, m, 0:4],
                                    lhsT=b2row[:, m * 128:(m + 1) * 128],
                                    rhs=ones1[:, 0:4],
                                    start=False, stop=False)
                        # critical: U-matmuls reading slot s+1
                        for m in range(KC):
                            for k in range(KC):
                                nc.tensor.matmul(
                                    out=P[:, m, 0:nccols],
                                    lhsT=u_sb[:, k, m * 128:(m + 1) * 128],
                                    rhs=he[:, k, s + 1, 0:nccols],
                                    start=False,
                                    stop=(m == KC - 1 and k == KC - 1))
                        nc.scalar.activation(
                            out=he[:, :, s + 2, 0:nccols],
                            in_=P[:, :, 0:nccols], func=AF.Tanh)
                        # H^T quarter transposes ride the encoder windows
                        if s in (34, 66, 98):
                            q = (s - 34) // 32
                            for b in range(BL):
                                for k in range(KC):
                                    tq = psm.tile([32, 128], DT, tag="tq", bufs=1)
                                    nc.tensor.matmul(
                                        out=tq[:, :],
                                        lhsT=he[:, k, 4 + 32 * q:36 + 32 * q, b],
                                        rhs=identb[:, :], start=True, stop=True)
                                    nc.vector.tensor_copy(
                                        out=ht_all[32 * q:32 * q + 32, b, k, :],
                                        in_=tq[:, :])


            # ---- decoder + blockwise attention + interleaved projection ----
            # Work items (attention sub-steps, out-projections, projection
            # groups) are fed one per decoder step into the tanh-wait windows.
            feed_hi = []   # attention + out-projection items
            feed_lo = []   # final-projection groups
            strip_tiles = {}

            in_tail = [False]

            def emit_proj_group(c, j):
                # token tile c: decoder steps [c*PBLK, (c+1)*PBLK) x BL batch
                R0 = c * PBLK
                pl = psL.tile([128, VC], DT, tag="pl")
                for k in range(KC):
                    nc.tensor.matmul(
                        out=pl[:, :],
                        lhsT=houts[:, k, R0:R0 + PBLK, :],
                        rhs=w_sb[:, k, j * VC:(j + 1) * VC],
                        start=(k == 0), stop=(k == KC - 1))
                q, r = divmod(j * VC, STRIP)
                stw = STRIP
                if r == 0:
                    strip_tiles[(c, q)] = lp.tile(
                        [128, STRIP], BF, tag="lt", name="lt", bufs=3)
                lt = strip_tiles[(c, q)]
                # PSUM->SBUF downcast split across DVE and ACT in parallel:
                # halves the PSUM-bank recycle latency (GPSIMD can't read PSUM)
                h = VC // 2
                nc.vector.tensor_copy(out=lt[:, r:r + h], in_=pl[:, 0:h])
                nc.scalar.copy(out=lt[:, r + h:r + VC], in_=pl[:, h:VC])
                if r + VC == stw:
                    nc.sync.dma_start(
                        out=out_d[c * 128:(c + 1) * 128, q * stw:(q + 1) * stw],
                        in_=lt[:, :])
                    del strip_tiles[(c, q)]

            attn_st = {}

            def emit_attn_scores(c, b):
                R = slice(1 + c * TBLK, 1 + (c + 1) * TBLK)  # decoder h slots
                S = psa.tile([TBLK, T], DT, tag="pa", name="S")
                for k in range(KC):
                    nc.tensor.matmul(
                        out=S[:, :], lhsT=hd[:, k, R, b],
                        rhs=he[:, k, 4:T + 4, b],
                        start=(k == 0), stop=False)
                nc.tensor.matmul(
                    out=S[:, :], lhsT=ones1[:, 0:TBLK],
                    rhs=maddb[0:1, b, :], start=False, stop=True)
                rmax = wp.tile([TBLK, 1], DT, tag="rmax")
                nc.vector.reduce_max(out=rmax[:, :], in_=S[:, :], axis=AX.X)
                nb = wp.tile([TBLK, 1], DT, tag="nb")
                nc.vector.tensor_scalar(
                    out=nb[:, :], in0=rmax[:, :], scalar1=-1.0 / 16.0,
                    scalar2=None, op0=ALU.mult)
                attn_st[(c, b)] = (S, nb)

            def emit_attn_softmax(c, b):
                S, nb = attn_st[(c, b)]
                exs = wp.tile([TBLK, T], DT, tag="exs")
                sums = wp.tile([TBLK, 1], DT, tag="sums")
                nc.scalar.activation(
                    out=exs[:, :], in_=S[:, :], func=AF.Exp,
                    bias=nb[:, :1], scale=1.0 / 16.0,
                    accum_out=sums[:, :1])
                rs = wp.tile([TBLK, 1], DT, tag="rs")
                nc.vector.reciprocal(out=rs[:, :], in_=sums[:, :])
                alb = wp.tile([TBLK, T], BF, tag="alb")
                nc.vector.tensor_scalar(
                    out=alb[:, :], in0=exs[:, :], scalar1=rs[:, :1],
                    scalar2=None, op0=ALU.mult)
                attn_st[(c, b)] = alb

            def emit_attn_transpose(c, b):
                alb = attn_st[(c, b)]
                pt = psa.tile([T, TBLK], DT, tag="pa", name="pt")
                nc.tensor.matmul(out=pt[:, :], lhsT=alb[:, :],
                                 rhs=identb[0:TBLK, 0:TBLK],
                                 start=True, stop=True)
                a_t = wp.tile([T, TBLK], BF, tag="a_t")
                nc.vector.tensor_copy(out=a_t[:, :], in_=pt[:, :])
                attn_st[(c, b)] = a_t

            def emit_attn_ctx(c, b):
                a_t = attn_st.pop((c, b))
                R0 = c * TBLK
                for m in range(KC):
                    pc = psa.tile([128, TBLK], DT, tag="pa", name="pc")
                    nc.tensor.matmul(
                        out=pc[:, :], lhsT=ht_all[:, b, m, :],
                        rhs=a_t[:, :], start=True, stop=True)
                    nc.vector.tensor_copy(
                        out=ctxs[:, m, R0:R0 + TBLK, b], in_=pc[:, :])

            def emit_outproj(c, m):
                # out' = h' + ctx_W @ ctx' for block c (TBLK*BL tokens)
                R = slice(1 + c * TBLK, 1 + (c + 1) * TBLK)
                R0 = c * TBLK
                po = psa.tile([128, TBLK, BL], DT, tag="pa", name="po")
                for k in range(KC):
                    nc.tensor.matmul(
                        out=po[:, :, :],
                        lhsT=cwt_sb[:, k, m * 128:(m + 1) * 128],
                        rhs=ctxs[:, k, R0:R0 + TBLK, :],
                        start=(k == 0), stop=(k == KC - 1))
                nc.vector.tensor_add(
                    out=houts[:, m, R0:R0 + TBLK, :],
                    in0=po[:, :, :], in1=hd[:, m, R, 0:4])
                if m == KC - 1:
                    for j in range(NVC):
                        feed_lo.append((emit_proj_group, (c, j)))

            with (
                tc.tile_pool(name="psd", bufs=2, space="PSUM") as psd,
                tc.tile_pool(name="psa", bufs=2, space="PSUM") as psa,
                tc.tile_pool(name="psL", bufs=4, space="PSUM") as psL,
            ):
                def emit_decx(b, k, q):
                    # transpose one 32-step quarter of decoder x into hd
                    tx = psa.tile([128, 32], DT, tag="pa", name="tx")
                    nc.tensor.matmul(
                        out=tx[:, :], lhsT=xgd[:, b, k * 128:(k + 1) * 128],
                        rhs=identb[:, 32 * q:32 * q + 32], start=True, stop=True)
                    nc.vector.tensor_copy(
                        out=hd[:, k, 32 * q:32 * q + 32, 4 + b], in_=tx[:, :])

                def emit_mask():
                    # maddb[b, t_src] = (src==0) * -1e9 (bf16 row)
                    mf = wp.tile([T, BL], DT, tag="mf")
                    nc.vector.tensor_copy(out=mf[:, :], in_=si_sb[:, :])
                    m01 = wp.tile([T, BL], DT, tag="m01")
                    nc.vector.tensor_scalar(
                        out=m01[:, :], in0=mf[:, :], scalar1=0.0, scalar2=None,
                        op0=ALU.is_equal)
                    mps = psa.tile([BL, T], DT, tag="pa", name="mps")
                    nc.tensor.matmul(out=mps[:, :], lhsT=m01[:, :],
                                     rhs=ident[:, :], start=True, stop=True)
                    ms4 = wp.tile([BL, T], BF, tag="ms4")
                    nc.vector.tensor_scalar(
                        out=ms4[:, :], in0=mps[:, :], scalar1=-1e9,
                        scalar2=None, op0=ALU.mult)
                    nc.sync.dma_start(out=maddb[0:1, :, :], in_=ms4[:, :])

                def emit_htq3(b, k):
                    # last H^T quarter (needs the final encoder act)
                    tq = psa.tile([32, 128], DT, tag="pa", name="tq")
                    nc.tensor.matmul(
                        out=tq[:, :], lhsT=he[:, k, 100:132, b],
                        rhs=identb[:, :], start=True, stop=True)
                    nc.vector.tensor_copy(
                        out=ht_all[96:128, b, k, :], in_=tq[:, :])

                for b in range(BL):
                    for k in range(KC):
                        emit_decx(b, k, 0)
                feed_hi.append((emit_mask, ()))
                for b in range(BL):
                    for k in range(KC):
                        feed_hi.append((emit_htq3, (b, k)))
                for q in range(1, 4):
                    for b in range(BL):
                        for k in range(KC):
                            feed_hi.append((emit_decx, (b, k, q)))

                for s in range(T):
                    P = psd.tile([128, KC, 4], DT, tag="pd")
                    # combined x idmm covers both chunks — prefetchable
                    nc.tensor.matmul(
                        out=P[:, :, :], lhsT=identb[:, :],
                        rhs=hd[:, :, s, 4:8], start=True, stop=False)
                    for m in range(KC):  # critical h-recurrence
                        for k in range(KC):
                            rhs = (he[:, k, T + 3, 0:4] if s == 0
                                   else hd[:, k, s, 0:4])
                            nc.tensor.matmul(
                                out=P[:, m, 0:4],
                                lhsT=u_sb[:, k, m * 128:(m + 1) * 128],
                                rhs=rhs, start=False,
                                stop=(m == KC - 1 and k == KC - 1))
                    nc.scalar.activation(
                        out=hd[:, :, s + 1, 0:4], in_=P[:, :, 0:4], func=AF.Tanh)
                    if s % TBLK == TBLK - 1:
                        c = s // TBLK
                        # stage-major emission so the 4 batch pipelines
                        # interleave on each engine queue
                        for st in (emit_attn_scores, emit_attn_softmax,
                                   emit_attn_transpose, emit_attn_ctx):
                            for b in range(BL):
                                st(c, b)
                        for m in range(KC):
                            emit_outproj(c, m)
                    if feed_hi:
                        fn, args = feed_hi.pop(0)
                        fn(*args)
                    else:
                        for _ in range(NLO):
                            if feed_lo:
                                fn, args = feed_lo.pop(0)
                                fn(*args)
                # tail: drain the remaining work items
                in_tail[0] = True
                while feed_lo:
                    fn, args = feed_lo.pop(0)
                    fn(*args)
    nc.compile()
    return nc


def _prep_in_maps(U, b_enc1, b_enc2, b_dec, E_en, E_de, ctx_W, W_out_de,
                  src_en, tgt_de_in):
    f32 = np.float32
    Ub = np.ascontiguousarray(np.asarray(U, f32)).astype(NPBF)
    ctx_wt = np.ascontiguousarray(np.asarray(ctx_W, f32).T).astype(NPBF)
    w_out_t = np.ascontiguousarray(np.asarray(W_out_de, f32).T).astype(NPBF)
    # fold per-layer input biases into the embedding tables (applied to
    # every token, PAD included — matches  x + b  inside the reference tanh)
    E_en = (np.asarray(E_en, f32) + np.asarray(b_enc1, f32)[None, :]).astype(NPBF)
    E_de = (np.asarray(E_de, f32) + np.asarray(b_dec, f32)[None, :]).astype(NPBF)
    b2row = np.ascontiguousarray(np.asarray(b_enc2, f32).reshape(1, D)).astype(NPBF)
    src = np.asarray(src_en).astype(np.int32)
    tgt = np.asarray(tgt_de_in).astype(np.int32)
    in_maps = []
    for i in range(NCORES):
        b0 = i * BL
        in_maps.append({
            "u": Ub, "ctx_wt": ctx_wt, "w_out_t": w_out_t,
            "e_en": E_en, "e_de": E_de, "b2row": b2row,
            "src_idx": np.ascontiguousarray(src[:, b0:b0 + BL]),
            "tgt_idx": np.ascontiguousarray(tgt[:, b0:b0 + BL]),
        })
    return in_maps


def kernel(U, b_enc1, b_enc2, b_dec, E_en, E_de, ctx_W, W_out_de,
           src_en, tgt_de_in, _trace=False, _raw=False):
    with_b2 = bool(np.any(np.asarray(b_enc2) != 0))
    key = ("nc", with_b2)
    if key not in _CACHE:
        _CACHE[key] = _build(with_b2)
    nc = _CACHE[key]
    in_maps = _prep_in_maps(U, b_enc1, b_enc2, b_dec, E_en, E_de, ctx_W,
                            W_out_de, src_en, tgt_de_in)
    res = run_bass_kernel_spmd(nc, in_maps, list(range(NCORES)), trace=_trace)
    if _raw:
        return res
    logits = np.empty((T, B, V), np.float32)
    for i in range(NCORES):
        logits[:, i * BL:(i + 1) * BL, :] = (
            res.results[i]["out"].astype(np.float32).reshape(T, BL, V))
    if _trace:
        return logits, res
    return logits


# revision 34
# speedup vs baseline: 1.0519x; 1.0039x over previous
"""Seq2seq RNN with attention on 8 TRN2 NeuronCores.

Strategy: data-parallel over batch. B=32 -> 4 batch elements per core.

Key ideas vs a naive per-step implementation:
- The decoder's h-recurrence does not depend on attention, so the decoder
  RNN runs as a bare tanh chain and attention/context/out-projection are
  computed afterwards in large batched matmuls (per 32-step block).
- RNN steps use a slot layout in SBUF: the single fused Tanh of step s
  writes [h2_{s-2} | h1_s] directly into slot s+2; the next step's
  matmuls read the slot directly (no copies on the serial chain). Layer-2
  runs 2 steps behind layer-1 so its h1-carry matmul is off the critical
  path. Per-step critical path = 4 U-matmuls + 1 activation.
- Biases b_enc1/b_dec are folded into the embedding tables host-side;
  b_enc2 is added with a prefetched rank-1 matmul (skipped when zero).
- The full W_out.T (16MB bf16) is DMA'd into SBUF during the encoder.
  Final-projection groups (2 matmuls of [128x500] + a copy) are
  interleaved one-per-decoder-step so they execute inside the tanh-wait
  windows of the serial chain; logits are staged in 16KB/partition strips
  and stored with few large DMAs.
"""

import numpy as np

import concourse.bass as bass
import concourse.bacc as bacc
import concourse.tile as tile
from concourse import mybir
from concourse.bass_utils import run_bass_kernel_spmd
from concourse.masks import make_identity

D = 256
V = 32000
T = 128  # T_SRC == T_TGT == 128
B = 32
NCORES = 8
BL = B // NCORES  # 4 batch elements per core
KC = D // 128  # 2 d-chunks of 128
DT = mybir.dt.float32
BF = mybir.dt.bfloat16
NPBF = mybir.dt.np(BF)
AF = mybir.ActivationFunctionType
ALU = mybir.AluOpType
AX = mybir.AxisListType

TBLK = 32            # decoder timesteps per attention block
NBLK = T // TBLK     # 4 blocks
PBLK = 32            # decoder timesteps per projection token tile (128 tokens)
VC = 500             # vocab chunk per projection matmul (psum <= 2KB/part)
NVC = V // VC        # 64 chunks per token tile
STRIP = 8000         # vocab columns per logits store strip (16KB/part bf16)
NSTRIP = V // STRIP  # 4 strips per token tile

_CACHE = {}


def _build(with_b2, NLO=1):
    nc = bacc.Bacc(None)

    u_d = nc.declare_dram_parameter("u", [D, D], BF, isOutput=False)
    cwt_d = nc.declare_dram_parameter("ctx_wt", [D, D], BF, isOutput=False)
    wot_d = nc.declare_dram_parameter("w_out_t", [D, V], BF, isOutput=False)
    een_d = nc.declare_dram_parameter("e_en", [V, D], BF, isOutput=False)
    ede_d = nc.declare_dram_parameter("e_de", [V, D], BF, isOutput=False)
    b2r_d = nc.declare_dram_parameter("b2row", [1, D], BF, isOutput=False)
    si_d = nc.declare_dram_parameter("src_idx", [T, BL], mybir.dt.int32, isOutput=False)
    ti_d = nc.declare_dram_parameter("tgt_idx", [T, BL], mybir.dt.int32, isOutput=False)
    out_d = nc.declare_dram_parameter("out", [T * BL, V], BF, isOutput=True)

    with tile.TileContext(nc) as tc:
        with (
            tc.tile_pool(name="persist", bufs=1) as pp,
            tc.tile_pool(name="work", bufs=6) as wp,
            tc.tile_pool(name="strips", bufs=3) as lp,
        ):
            # ---- persistent SBUF tiles ----
            u_sb = pp.tile([128, KC, D], BF, tag="u")
            cwt_sb = pp.tile([128, KC, D], BF, tag="cwt")
            w_sb = pp.tile([128, KC, V], BF, tag="w")  # 128KB/partition
            ident = pp.tile([128, 128], DT, tag="ident")
            identb = pp.tile([128, 128], BF, tag="identb")
            ones1 = pp.tile([1, 128], BF, tag="ones1")
            b2row = pp.tile([1, D], BF, tag="b2row")
            si_sb = pp.tile([T, BL], mybir.dt.int32, tag="si")
            ti_sb = pp.tile([T, BL], mybir.dt.int32, tag="ti")
            maddb = pp.tile([1, BL, T], BF, tag="maddb")
            # encoder slots: he[:, k, s, 0:4]=h2-field, 4:8=h1-field, 8:12=x-field
            # act of iter s writes he[:,:,s+2,0:8] = [h2_{s-2} | h1_s]
            # x_t lives at he[:, k, t+1, 8:12]; H[t]=h2_t at he[:, k, t+4, 0:4]
            he = pp.tile([128, KC, T + 4, 12], BF, tag="he")
            # decoder slots: hd[:, k, s, 0:4]=h-field (h_{s-1}), 4:8=x-field (x_s)
            hd = pp.tile([128, KC, T + 1, 8], BF, tag="hd")
            ht_all = pp.tile([128, BL, KC, 128], BF, tag="ht")  # H_b^T [t,b,k,d]
            ctxs = pp.tile([128, KC, T, BL], BF, tag="ctxs")    # ctx' [d,k,t,b]
            houts = pp.tile([128, KC, T, BL], BF, tag="houts")  # outs' [d,k,t,b]

            # ---- small loads: indices first (they gate the gathers that
            # gate the whole RNN chain), weights after ----
            nc.sync.dma_start(out=si_sb[:, :], in_=si_d[:, :])
            nc.sync.dma_start(out=ti_sb[:, :], in_=ti_d[:, :])
            for k in range(KC):
                nc.sync.dma_start(out=u_sb[:, k, :], in_=u_d[k * 128:(k + 1) * 128, :])
                nc.sync.dma_start(out=cwt_sb[:, k, :],
                                  in_=cwt_d[k * 128:(k + 1) * 128, :])
            nc.sync.dma_start(out=b2row[:, :], in_=b2r_d[:, :])

            make_identity(nc, ident[:, :])
            nc.vector.tensor_copy(out=identb[:, :], in_=ident[:, :])
            nc.vector.memset(ones1[:, :], 1.0)
            # zero-init slots read before first writes
            nc.vector.memset(he[:, :, 0, :], 0.0)
            nc.vector.memset(he[:, :, 1, 0:8], 0.0)
            # x-fields of the two encoder tail slots are read (never used)
            # by the uniform combined idmm — keep them finite
            nc.vector.memset(he[:, :, T, 8:12], 0.0)
            nc.vector.memset(he[:, :, T + 1, 8:12], 0.0)

            with tc.tile_pool(name="psm", bufs=2, space="PSUM") as psm:
                # ---- gather embeddings; transpose encoder x to [d, t] now,
                # decoder x is transposed later (quarters fed into decoder
                # tanh-wait windows) so the encoder chain starts sooner ----
                xge = wp.tile([T, BL, D], BF, tag="xg", bufs=2)
                for b in range(BL):
                    nc.gpsimd.indirect_dma_start(
                        out=xge[:, b, :], out_offset=None, in_=een_d[:, :],
                        in_offset=bass.IndirectOffsetOnAxis(
                            ap=si_sb[:, b:b + 1], axis=0))
                xgd = wp.tile([T, BL, D], BF, tag="xg", bufs=2)
                for b in range(BL):
                    nc.gpsimd.indirect_dma_start(
                        out=xgd[:, b, :], out_offset=None, in_=ede_d[:, :],
                        in_offset=bass.IndirectOffsetOnAxis(
                            ap=ti_sb[:, b:b + 1], axis=0))
                for b in range(BL):
                    for k in range(KC):
                        tp = psm.tile([128, 128], DT, tag="tp")
                        nc.tensor.matmul(
                            out=tp[:, :],
                            lhsT=xge[:, b, k * 128:(k + 1) * 128],
                            rhs=identb[:, :], start=True, stop=True)
                        nc.vector.tensor_copy(
                            out=he[:, k, 0:T, 8 + b], in_=tp[:, :])

                # W_out prefetch after the gathers. The shared DMA fabric is
                # FCFS, so gate each chunk behind the last gather with a
                # 1-element dummy write (WAW dep) — otherwise the big W
                # transfers win the fabric and stall the RNN start by ~50us.
                for k in range(KC):
                    for q in range(4):
                        nc.vector.tensor_copy(
                            out=w_sb[0:1, k, q * 8000:q * 8000 + 1],
                            in_=xgd[0:1, 0, 0:1])
                        nc.sync.dma_start(
                            out=w_sb[:, k, q * 8000:(q + 1) * 8000],
                            in_=wot_d[k * 128:(k + 1) * 128,
                                      q * 8000:(q + 1) * 8000])

                # ---- encoder: iterations s = 0 .. T+1 ----
                # P[:, m, 0:4] = h2-part: U^T h2_{s-3} + h1_{s-2} (+ b2)
                # P[:, m, 4:8] = h1-part: U^T h1_{s-1} + x_s      (s < T)
                with tc.tile_pool(name="pse", bufs=2, space="PSUM") as pse:
                    for s in range(T + 2):
                        h1p = s < T  # compute the h1-part this iteration?
                        nccols = 8 if h1p else 4
                        # one 2KB bank (= zero region) per d-chunk m so each
                        # chunk's accumulation group starts with a matmul that
                        # covers every column it will ever touch
                        P = pse.tile([128, KC, 8], DT, tag="pe")
                        # one idmm covers every column the group will touch:
                        # [h1-carry | x] for both d-chunks (contiguous out AP)
                        nc.tensor.matmul(
                            out=P[:, :, 0:8], lhsT=identb[:, :],
                            rhs=he[:, :, s, 4:12],
                            start=True, stop=False)
                        if with_b2:
                            for m in range(KC):
                                nc.tensor.matmul(
                                    out=P[:, m, 0:4],
                                    lhsT=b2row[:, m * 128:(m + 1) * 128],
                                    rhs=ones1[:, 0:4],
                                    start=False, stop=False)
                        # critical: U-matmuls reading slot s+1
                        for m in range(KC):
                            for k in range(KC):
                                nc.tensor.matmul(
                                    out=P[:, m, 0:nccols],
                                    lhsT=u_sb[:, k, m * 128:(m + 1) * 128],
                                    rhs=he[:, k, s + 1, 0:nccols],
                                    start=False,
                                    stop=(m == KC - 1 and k == KC - 1))
                        nc.scalar.activation(
                            out=he[:, :, s + 2, 0:nccols],
                            in_=P[:, :, 0:nccols], func=AF.Tanh)
                        # H^T quarter transposes ride the encoder windows
                        if s in (34, 66, 98):
                            q = (s - 34) // 32
                            for b in range(BL):
                                for k in range(KC):
                                    tq = psm.tile([32, 128], DT, tag="tq", bufs=1)
                                    nc.tensor.matmul(
                                        out=tq[:, :],
                                        lhsT=he[:, k, 4 + 32 * q:36 + 32 * q, b],
                                        rhs=identb[:, :], start=True, stop=True)
                                    nc.vector.tensor_copy(
                                        out=ht_all[32 * q:32 * q + 32, b, k, :],
                                        in_=tq[:, :])


            # ---- decoder + blockwise attention + interleaved projection ----
            # Work items (attention sub-steps, out-projections, projection
            # groups) are fed one per decoder step into the tanh-wait windows.
            feed_hi = []   # attention + out-projection items
            feed_lo = []   # final-projection groups
            strip_tiles = {}

            in_tail = [False]

            def emit_proj_group(c, j):
                # token tile c: decoder steps [c*PBLK, (c+1)*PBLK) x BL batch
                R0 = c * PBLK
                pl = psL.tile([128, VC], DT, tag="pl")
                for k in range(KC):
                    nc.tensor.matmul(
                        out=pl[:, :],
                        lhsT=houts[:, k, R0:R0 + PBLK, :],
                        rhs=w_sb[:, k, j * VC:(j + 1) * VC],
                        start=(k == 0), stop=(k == KC - 1))
                q, r = divmod(j * VC, STRIP)
                stw = STRIP
                if r == 0:
                    strip_tiles[(c, q)] = lp.tile(
                        [128, STRIP], BF, tag="lt", name="lt", bufs=3)
                lt = strip_tiles[(c, q)]
                # PSUM->SBUF downcast split across DVE and ACT in parallel:
                # halves the PSUM-bank recycle latency (GPSIMD can't read PSUM)
                h = VC // 2
                nc.vector.tensor_copy(out=lt[:, r:r + h], in_=pl[:, 0:h])
                nc.scalar.copy(out=lt[:, r + h:r + VC], in_=pl[:, h:VC])
                last = c == NBLK - 1 and q == NSTRIP - 1
                piece = stw // 4 if last else stw
                if (r + VC) % piece == 0:
                    p0 = (r + VC) - piece
                    nc.sync.dma_start(
                        out=out_d[c * 128:(c + 1) * 128,
                                  q * stw + p0:q * stw + p0 + piece],
                        in_=lt[:, p0:p0 + piece])
                if r + VC == stw:
                    del strip_tiles[(c, q)]

            attn_st = {}

            def emit_attn_scores(c, b):
                R = slice(1 + c * TBLK, 1 + (c + 1) * TBLK)  # decoder h slots
                S = psa.tile([TBLK, T], DT, tag="pa", name="S")
                for k in range(KC):
                    nc.tensor.matmul(
                        out=S[:, :], lhsT=hd[:, k, R, b],
                        rhs=he[:, k, 4:T + 4, b],
                        start=(k == 0), stop=False)
                nc.tensor.matmul(
                    out=S[:, :], lhsT=ones1[:, 0:TBLK],
                    rhs=maddb[0:1, b, :], start=False, stop=True)
                rmax = wp.tile([TBLK, 1], DT, tag="rmax")
                nc.vector.reduce_max(out=rmax[:, :], in_=S[:, :], axis=AX.X)
                nb = wp.tile([TBLK, 1], DT, tag="nb")
                nc.vector.tensor_scalar(
                    out=nb[:, :], in0=rmax[:, :], scalar1=-1.0 / 16.0,
                    scalar2=None, op0=ALU.mult)
                attn_st[(c, b)] = (S, nb)

            def emit_attn_softmax(c, b):
                S, nb = attn_st[(c, b)]
                exs = wp.tile([TBLK, T], DT, tag="exs")
                sums = wp.tile([TBLK, 1], DT, tag="sums")
                nc.scalar.activation(
                    out=exs[:, :], in_=S[:, :], func=AF.Exp,
                    bias=nb[:, :1], scale=1.0 / 16.0,
                    accum_out=sums[:, :1])
                rs = wp.tile([TBLK, 1], DT, tag="rs")
                nc.vector.reciprocal(out=rs[:, :], in_=sums[:, :])
                alb = wp.tile([TBLK, T], BF, tag="alb")
                nc.vector.tensor_scalar(
                    out=alb[:, :], in0=exs[:, :], scalar1=rs[:, :1],
                    scalar2=None, op0=ALU.mult)
                attn_st[(c, b)] = alb

            def emit_attn_transpose(c, b):
                alb = attn_st[(c, b)]
                pt = psa.tile([T, TBLK], DT, tag="pa", name="pt")
                nc.tensor.matmul(out=pt[:, :], lhsT=alb[:, :],
                                 rhs=identb[0:TBLK, 0:TBLK],
                                 start=True, stop=True)
                a_t = wp.tile([T, TBLK], BF, tag="a_t")
                nc.vector.tensor_copy(out=a_t[:, :], in_=pt[:, :])
                attn_st[(c, b)] = a_t

            def emit_attn_ctx(c, b):
                a_t = attn_st.pop((c, b))
                R0 = c * TBLK
                for m in range(KC):
                    pc = psa.tile([128, TBLK], DT, tag="pa", name="pc")
                    nc.tensor.matmul(
                        out=pc[:, :], lhsT=ht_all[:, b, m, :],
                        rhs=a_t[:, :], start=True, stop=True)
                    nc.vector.tensor_copy(
                        out=ctxs[:, m, R0:R0 + TBLK, b], in_=pc[:, :])

            def emit_outproj(c, m):
                # out' = h' + ctx_W @ ctx' for block c (TBLK*BL tokens)
                R = slice(1 + c * TBLK, 1 + (c + 1) * TBLK)
                R0 = c * TBLK
                po = psa.tile([128, TBLK, BL], DT, tag="pa", name="po")
                for k in range(KC):
                    nc.tensor.matmul(
                        out=po[:, :, :],
                        lhsT=cwt_sb[:, k, m * 128:(m + 1) * 128],
                        rhs=ctxs[:, k, R0:R0 + TBLK, :],
                        start=(k == 0), stop=(k == KC - 1))
                nc.vector.tensor_add(
                    out=houts[:, m, R0:R0 + TBLK, :],
                    in0=po[:, :, :], in1=hd[:, m, R, 0:4])
                if m == KC - 1:
                    for j in range(NVC):
                        feed_lo.append((emit_proj_group, (c, j)))

            with (
                tc.tile_pool(name="psd", bufs=2, space="PSUM") as psd,
                tc.tile_pool(name="psa", bufs=2, space="PSUM") as psa,
                tc.tile_pool(name="psL", bufs=4, space="PSUM") as psL,
            ):
                def emit_decx(b, k, q):
                    # transpose one 32-step quarter of decoder x into hd
                    tx = psa.tile([128, 32], DT, tag="pa", name="tx")
                    nc.tensor.matmul(
                        out=tx[:, :], lhsT=xgd[:, b, k * 128:(k + 1) * 128],
                        rhs=identb[:, 32 * q:32 * q + 32], start=True, stop=True)
                    nc.vector.tensor_copy(
                        out=hd[:, k, 32 * q:32 * q + 32, 4 + b], in_=tx[:, :])

                def emit_mask():
                    # maddb[b, t_src] = (src==0) * -1e9 (bf16 row)
                    mf = wp.tile([T, BL], DT, tag="mf")
                    nc.vector.tensor_copy(out=mf[:, :], in_=si_sb[:, :])
                    m01 = wp.tile([T, BL], DT, tag="m01")
                    nc.vector.tensor_scalar(
                        out=m01[:, :], in0=mf[:, :], scalar1=0.0, scalar2=None,
                        op0=ALU.is_equal)
                    mps = psa.tile([BL, T], DT, tag="pa", name="mps")
                    nc.tensor.matmul(out=mps[:, :], lhsT=m01[:, :],
                                     rhs=ident[:, :], start=True, stop=True)
                    ms4 = wp.tile([BL, T], BF, tag="ms4")
                    nc.vector.tensor_scalar(
                        out=ms4[:, :], in0=mps[:, :], scalar1=-1e9,
                        scalar2=None, op0=ALU.mult)
                    nc.sync.dma_start(out=maddb[0:1, :, :], in_=ms4[:, :])

                def emit_htq3(b, k):
                    # last H^T quarter (needs the final encoder act)
                    tq = psa.tile([32, 128], DT, tag="pa", name="tq")
                    nc.tensor.matmul(
                        out=tq[:, :], lhsT=he[:, k, 100:132, b],
                        rhs=identb[:, :], start=True, stop=True)
                    nc.vector.tensor_copy(
                        out=ht_all[96:128, b, k, :], in_=tq[:, :])

                for b in range(BL):
                    for k in range(KC):
                        emit_decx(b, k, 0)
                feed_hi.append((emit_mask, ()))
                for b in range(BL):
                    for k in range(KC):
                        feed_hi.append((emit_htq3, (b, k)))
                for q in range(1, 4):
                    for b in range(BL):
                        for k in range(KC):
                            feed_hi.append((emit_decx, (b, k, q)))

                for s in range(T):
                    P = psd.tile([128, KC, 4], DT, tag="pd")
                    # combined x idmm covers both chunks — prefetchable
                    nc.tensor.matmul(
                        out=P[:, :, :], lhsT=identb[:, :],
                        rhs=hd[:, :, s, 4:8], start=True, stop=False)
                    for m in range(KC):  # critical h-recurrence
                        for k in range(KC):
                            rhs = (he[:, k, T + 3, 0:4] if s == 0
                                   else hd[:, k, s, 0:4])
                            nc.tensor.matmul(
                                out=P[:, m, 0:4],
                                lhsT=u_sb[:, k, m * 128:(m + 1) * 128],
                                rhs=rhs, start=False,
                                stop=(m == KC - 1 and k == KC - 1))
                    nc.scalar.activation(
                        out=hd[:, :, s + 1, 0:4], in_=P[:, :, 0:4], func=AF.Tanh)
                    if s % TBLK == TBLK - 1:
                        c = s // TBLK
                        # stage-major emission so the 4 batch pipelines
                        # interleave on each engine queue
                        for st in (emit_attn_scores, emit_attn_softmax,
                                   emit_attn_transpose, emit_attn_ctx):
                            for b in range(BL):
                                st(c, b)
                        for m in range(KC):
                            emit_outproj(c, m)
                    if feed_hi:
                        fn, args = feed_hi.pop(0)
                        fn(*args)
                    else:
                        for _ in range(NLO):
                            if feed_lo:
                                fn, args = feed_lo.pop(0)
                                fn(*args)
                # tail: drain the remaining work items
                in_tail[0] = True
                while feed_lo:
                    fn, args = feed_lo.pop(0)
                    fn(*args)
    nc.compile()
    return nc


def _prep_in_maps(U, b_enc1, b_enc2, b_dec, E_en, E_de, ctx_W, W_out_de,
                  src_en, tgt_de_in):
    f32 = np.float32
    Ub = np.ascontiguousarray(np.asarray(U, f32)).astype(NPBF)
    ctx_wt = np.ascontiguousarray(np.asarray(ctx_W, f32).T).astype(NPBF)
    w_out_t = np.ascontiguousarray(np.asarray(W_out_de, f32).T).astype(NPBF)
    # fold per-layer input biases into the embedding tables (applied to
    # every token, PAD included — matches  x + b  inside the reference tanh)
    E_en = (np.asarray(E_en, f32) + np.asarray(b_enc1, f32)[None, :]).astype(NPBF)
    E_de = (np.asarray(E_de, f32) + np.asarray(b_dec, f32)[None, :]).astype(NPBF)
    b2row = np.ascontiguousarray(np.asarray(b_enc2, f32).reshape(1, D)).astype(NPBF)
    src = np.asarray(src_en).astype(np.int32)
    tgt = np.asarray(tgt_de_in).astype(np.int32)
    in_maps = []
    for i in range(NCORES):
        b0 = i * BL
        in_maps.append({
            "u": Ub, "ctx_wt": ctx_wt, "w_out_t": w_out_t,
            "e_en": E_en, "e_de": E_de, "b2row": b2row,
            "src_idx": np.ascontiguousarray(src[:, b0:b0 + BL]),
            "tgt_idx": np.ascontiguousarray(tgt[:, b0:b0 + BL]),
        })
    return in_maps


def kernel(U, b_enc1, b_enc2, b_dec, E_en, E_de, ctx_W, W_out_de,
           src_en, tgt_de_in, _trace=False, _raw=False):
    with_b2 = bool(np.any(np.asarray(b_enc2) != 0))
    key = ("nc", with_b2)
    if key not in _CACHE:
        _CACHE[key] = _build(with_b2)
    nc = _CACHE[key]
    in_maps = _prep_in_maps(U, b_enc1, b_enc2, b_dec, E_en, E_de, ctx_W,
                            W_out_de, src_en, tgt_de_in)
    res = run_bass_kernel_spmd(nc, in_maps, list(range(NCORES)), trace=_trace)
    if _raw:
        return res
    logits = np.empty((T, B, V), np.float32)
    for i in range(NCORES):
        logits[:, i * BL:(i + 1) * BL, :] = (
            res.results[i]["out"].astype(np.float32).reshape(T, BL, V))
    if _trace:
        return logits, res
    return logits
